# revision 1
# baseline (speedup 1.0000x reference)
"""Trainium2 Bass kernel for nn_KDHR (gnn_message_passing).

Math reduction: with S[d,s] = #edges (s->d) over N_SH=1195 nodes, each
GCN-mean layer is h = tanh(Sn @ (x @ W.T) + b), where Sn = S / max(cnt,1)
is row-normalized on the HOST (counts built once from the edge list).
W1 is also folded on the host (x1w = SH_emb @ W1.T), as is the row-norm
of the embedding (x1n).  The mlp is folded into es (es2 = es @ mlp_W.T)
and mlp_b cancels inside BatchNorm, so the device only runs:

  L1:   h1T = tanh(x1w^T @ SnT + b1)            (bf16 matmuls)
  L2:   h1w = h1 @ W2.T (per 128-chunk, fp32r)  -> h2T = tanh(h1w^T @ SnT + b2)
  es/eh: col-norm scales + host row-norm add
  batch: zT = (es2n^T @ X) * recip(ones^T @ X)  (X = P^T in bf16)
  BN:   stats all-reduced ([64,2]) -> zbn = relu(zT*s + t)
  out:  per 128-row tile: zbn_chunk^T @ ehT -> bf16 -> DRAM

All big matmuls stream bf16 or fp32r (1 cycle/row); batch (16384) is
sharded 2048 rows/core across 8 cores.
"""

import os
import sys

for _p in ("/root/.axon_site", "/root/.axon_site/_ro/trn_rl_repo",
           "/root/.axon_site/_ro/pypackages", "/opt/trn_rl_repo", "/opt/pypackages"):
    if os.path.isdir(_p) and _p not in sys.path:
        sys.path.append(_p)

import numpy as np

import concourse.bass as bass
import concourse.mybir as mybir
import concourse.tile as tile
from concourse import bacc
from concourse.bass_utils import run_bass_kernel_spmd

N_USER, N_ITEM, N_SH, D = 805, 390, 1195, 64
B, NCORES = 16384, 8
BS = B // NCORES          # 2048 batch rows per core
NKC = 10                  # source-node chunks (1195 padded to 1280)
NPAD = NKC * 128
BN_EPS = 1e-5
NORM_EPS = 1e-12
F32 = mybir.dt.float32
F32R = mybir.dt.float32r
BF16 = mybir.dt.bfloat16

AG_NSL = [(0, 512), (512, 512), (1024, 171)]     # at/bt col chunks (PSUM banks)
OUT_NSL = [(0, 512), (512, 293)]                 # out col chunks
NQ = 4
QW = BS // NQ                                    # 512
# stn DMA groups of k-chunks (pipelines L1 behind the loads)
GR = [(0, 3), (3, 3), (6, 3), (9, 1)]
# params tensor column layout
PAR_X1N, PAR_VEC = 0, 1195
WARM2 = 0
PAR_W = 1199  # x1nT(1195) | b1,b2,gamma,beta(4)


def _build(collective=True):
    nc = bacc.Bacc("TRN2", target_bir_lowering=False, debug=False,
                   num_devices=NCORES)

    xp = nc.declare_dram_parameter("xp", [128, 3, BS], BF16, isOutput=False).ap()
    xp3 = nc.declare_dram_parameter("xp3", [6, BS], BF16, isOutput=False).ap()
    stn = nc.declare_dram_parameter("stn", [128, NKC, N_SH], BF16, isOutput=False).ap()
    x1w = nc.declare_dram_parameter("x1w", [128, NKC, D], BF16, isOutput=False).ap()
    par = nc.declare_dram_parameter("par", [D, PAR_W], F32, isOutput=False).ap()
    wts = nc.declare_dram_parameter("wts", [D, 2 * D], BF16, isOutput=False).ap()
    out = nc.declare_dram_parameter("out", [128, BS // 128, N_USER], BF16,
                                    isOutput=True).ap()

    from contextlib import ExitStack
    with tile.TileContext(nc) as tc, ExitStack() as ctx:
        pools = {
            "cst": ctx.enter_context(tc.tile_pool(name="cst", bufs=1)),
            "sb": ctx.enter_context(tc.tile_pool(name="sb", bufs=1)),
            "scr": ctx.enter_context(tc.tile_pool(name="scr", bufs=2)),
            "outp": ctx.enter_context(tc.tile_pool(name="outp", bufs=4)),
            "psA": ctx.enter_context(tc.tile_pool(name="psA", bufs=1, space="PSUM")),
            "psT": ctx.enter_context(tc.tile_pool(name="psT", bufs=2, space="PSUM")),
            "dram": ctx.enter_context(tc.tile_pool(name="dram", bufs=1, space="DRAM")),
        }
        _body(nc, tc, pools, xp, xp3, stn, x1w, par, wts, out, collective)

    nc.compile()
    return nc


def _body(nc, tc, P, xp, xp3, stn, x1w, par, wts, out, collective=True):
    AF = mybir.ActivationFunctionType
    ALU = mybir.AluOpType
    AX = mybir.AxisListType
    cst, sb, scr, outp = P["cst"], P["sb"], P["scr"], P["outp"]
    psA, psT, dram = P["psA"], P["psT"], P["dram"]

    # ---- constants / parameters ----
    ones = cst.tile([128, D], BF16, tag="ones")
    nc.vector.memset(ones[:], 1.0)
    epst = cst.tile([D, 1], F32, tag="epst")
    nc.vector.memset(epst[:], BN_EPS)
    # touch the tanh act-func set at t=0 so the 1.28us LoadActFuncSet
    # happens while Act is otherwise idle, not before the first real tanh
    warmact = cst.tile([D, 1], F32, tag="warmact")
    with tc.high_priority():
        nc.scalar.activation(warmact[:], epst[:], AF.Tanh)

    x1w_sb = cst.tile([128, NKC, D], BF16, tag="x1w")
    nc.sync.dma_start(x1w_sb[:], x1w[:, :, :])

    stg = []
    for gi, (g0, gn) in enumerate(GR):
        t = sb.tile([128, gn, N_SH], BF16, tag=f"stn{gi}", name=f"stn{gi}")
        nc.sync.dma_start(t[:], stn[:, g0:g0 + gn, :])
        stg.append(t)

    par_sb = cst.tile([D, PAR_W], F32, tag="par")
    nc.sync.dma_start(par_sb[:], par[:, :])
    b1 = par_sb[:, PAR_VEC + 0:PAR_VEC + 1]
    b2 = par_sb[:, PAR_VEC + 1:PAR_VEC + 2]
    gam = par_sb[:, PAR_VEC + 2:PAR_VEC + 3]
    bet = par_sb[:, PAR_VEC + 3:PAR_VEC + 4]
    wts_sb = cst.tile([D, 2 * D], BF16, tag="wts")
    nc.sync.dma_start(wts_sb[:], wts[:, :])
    w2b = wts_sb[:, 0:D]
    mwb = wts_sb[:, D:2 * D]

    X = sb.tile([128, 3, BS], BF16, tag="X")
    nc.sync.dma_start(X[:], xp[:, :, :])
    X3 = sb.tile([6, BS], BF16, tag="X3")
    nc.sync.dma_start(X3[:], xp3[:, :])

    def st_chunk(k, c0, cn):
        gi, kl = (3, k - 9) if k >= 9 else (k // 3, k % 3)
        return stg[gi][:, kl, c0:c0 + cn]

    # ---- L1: atT = x1w^T @ SnT, chunk-pipelined behind the stn DMAs ----
    at = psA.tile([D, N_SH], F32, tag="ag")
    for k in range(NKC):
        for c0, cn in AG_NSL:
            nc.tensor.matmul(at[:, c0:c0 + cn], x1w_sb[:, k, :], st_chunk(k, c0, cn),
                             start=(k == 0), stop=(k == NKC - 1))
    h1t = sb.tile([D, NPAD], BF16, tag="h1t")
    nc.vector.memset(h1t[:, N_SH:NPAD], 0.0)
    for c0, cn in AG_NSL:
        nc.scalar.activation(h1t[:, c0:c0 + cn], at[:, c0:c0 + cn], AF.Tanh,
                             bias=b1)

    # ---- L2 prep: h1w_k = h1[128-chunk] @ W2.T (bf16), stored bf16 ----
    h1w = []
    for k in range(NKC):
        tp = psT.tile([128, D], F32, tag="tr", bufs=1)
        nc.tensor.matmul(tp[:], h1t[:, 128 * k:128 * (k + 1)], w2b,
                         start=True, stop=True)
        hb = sb.tile([128, D], BF16, tag=f"h1w{k}", name=f"h1w{k}")
        nc.vector.tensor_copy(hb[:], tp[:])
        h1w.append(hb)

    # ---- L2: btT = h1w^T @ SnT ----
    bt = psA.tile([D, N_SH], F32, tag="ag")
    for k in range(NKC):
        for c0, cn in AG_NSL:
            nc.tensor.matmul(bt[:, c0:c0 + cn], h1w[k][:], st_chunk(k, c0, cn),
                             start=(k == 0), stop=(k == NKC - 1))
    h2t = sb.tile([D, N_SH], F32, tag="h2t")
    for c0, cn in AG_NSL:
        nc.scalar.activation(h2t[:, c0:c0 + cn], bt[:, c0:c0 + cn], AF.Tanh,
                             bias=b2)

    # ---- presum: raw row-sums of P, replicated over 64 partitions ----
    # (PE streams X once with an all-ones stationary; recip on DVE)
    rp_sb = sb.tile([D, BS], F32, tag="rp_sb")
    for q in range(NQ):
        t = psT.tile([D, QW], F32, tag="oL", name=f"rp{q}")
        for c in range(3):
            nc.tensor.matmul(t[:], ones[:], X[:, c, q * QW:(q + 1) * QW],
                             start=(c == 0), stop=False)
        nc.tensor.matmul(t[:], ones[:6, :], X3[:, q * QW:(q + 1) * QW],
                         start=False, stop=True)
        nc.vector.reciprocal(rp_sb[:, q * QW:(q + 1) * QW], t[:])

    # ---- col norms of h2 (user/item) -> rcu = 1/sqrt(sum h2^2) ----
    sq_scr = sb.tile([D, N_USER], F32, tag="sq_scr")
    rc = sb.tile([D, 4], F32, tag="rc")
    nc.scalar.activation(sq_scr[:, 0:N_ITEM], h2t[:, N_USER:N_SH], AF.Square,
                         accum_out=rc[:, 1:2])
    nc.scalar.activation(rc[:, 3:4], rc[:, 1:2], AF.Sqrt)
    nc.vector.reciprocal(rc[:, 3:4], rc[:, 3:4])
    # ---- esT first (feeds es2n/esy); eh built later (only gates out) ----
    esf = sb.tile([D, N_ITEM], F32, tag="esf")
    nc.scalar.activation(esf[:], h2t[:, N_USER:N_SH], AF.Copy, scale=rc[:, 3:4])
    est = sb.tile([D, N_ITEM], BF16, tag="est")
    nc.vector.tensor_add(est[:], esf[:],
                         par_sb[:, PAR_X1N + N_USER:PAR_X1N + N_SH])

    # ---- PE warm-keeper: harmless filler matmuls over X into a scratch
    # PSUM bank while the es chain (Act/DVE) runs, so the tensor engine's
    # clock stays ramped for esy/out ----
    warm = psT.tile([D, QW], F32, tag="oL", name="warm")
    for w in range(6):
        nc.tensor.matmul(warm[:], ones[:], X[:, w % 3, 0:QW],
                         start=(w == 0), stop=(w == 5))

    # ---- es2n chunks: es2 = es @ mlp_W.T, natural layout, bf16 ----
    es2n = []
    for c in range(4):
        c0 = 128 * c
        cn = min(128, N_ITEM - c0)
        tp = psT.tile([128, D], F32, tag=("tr" if c % 2 == 0 else "oL"),
                      bufs=(1 if c % 2 == 0 else 2), name=f"es2p{c}")
        nc.tensor.matmul(tp[:cn, :], est[:, c0:c0 + cn], mwb,
                         start=True, stop=True)
        eb = sb.tile([128, D], BF16, tag=f"es2n{c}", name=f"es2n{c}")
        nc.vector.tensor_copy(eb[:cn, :], tp[:cn, :])
        es2n.append((eb, cn))

    # ---- esy quarters -> zT = esy * 1/presum;  BN partial sums chase ----
    zt = sb.tile([D, BS], F32, tag="zt")
    s12 = sb.tile([D, 2 * NQ], F32, tag="s12")
    for q in range(NQ):
        t = psT.tile([D, QW], F32, tag="oR", name=f"esy{q}")
        for c in range(4):
            eb, cn = es2n[c]
            rhs = (X[:, c, q * QW:(q + 1) * QW] if c < 3
                   else X3[:, q * QW:(q + 1) * QW])
            nc.tensor.matmul(t[:], eb[:cn, :], rhs, start=(c == 0), stop=(c == 3))
        ztq = zt[:, q * QW:(q + 1) * QW]
        nc.vector.tensor_mul(ztq, t[:], rp_sb[:, q * QW:(q + 1) * QW])
        nc.vector.tensor_reduce(s12[:, q:q + 1], ztq, axis=AX.X, op=ALU.add)
        sq = scr.tile([D, QW], F32, tag="sq")
        nc.scalar.activation(sq[:], ztq, AF.Square,
                             accum_out=s12[:, NQ + q:NQ + q + 1])

    # ---- user-side norm + ehT (gates only the out stage) ----
    nc.scalar.activation(sq_scr[:, 0:N_USER], h2t[:, 0:N_USER], AF.Square,
                         accum_out=rc[:, 0:1])
    nc.scalar.activation(rc[:, 2:3], rc[:, 0:1], AF.Sqrt)
    nc.vector.reciprocal(rc[:, 2:3], rc[:, 2:3])
    ehf = sb.tile([D, N_USER], F32, tag="ehf")
    nc.scalar.activation(ehf[:], h2t[:, 0:N_USER], AF.Copy, scale=rc[:, 2:3])
    eht = sb.tile([D, N_USER], BF16, tag="eht")
    nc.gpsimd.tensor_add(eht[:], ehf[:], par_sb[:, PAR_X1N:PAR_X1N + N_USER])

    stats = sb.tile([D, 2], F32, tag="stats")
    nc.vector.tensor_reduce(stats[:, 0:1], s12[:, 0:NQ], axis=AX.X, op=ALU.add)
    nc.vector.tensor_reduce(stats[:, 1:2], s12[:, NQ:2 * NQ], axis=AX.X, op=ALU.add)

    # ---- PE warm-keeper #2: bridge the allreduce window so the first out
    # matmuls run at full clock ----
    if WARM2 > 0:
        warm2 = psT.tile([D, QW], F32, tag="oL", name="warm2")
        for w in range(WARM2):
            nc.tensor.matmul(warm2[:], ones[:], X[:, w % 3, 0:QW],
                             start=(w == 0), stop=(w == WARM2 - 1))

    # ---- all-reduce BN stats ([64,2]) ----
    st_in = dram.tile([D, 2], F32, tag="cc_in")
    st_out = dram.tile([D, 2], F32, tag="cc_out")
    nc.sync.dma_start(st_in[:], stats[:])
    if collective:
        nc.gpsimd.collective_compute(
            "AllReduce", mybir.AluOpType.add,
            replica_groups=[list(range(NCORES))],
            ins=[st_in.opt()], outs=[st_out.opt()])
    else:
        nc.sync.dma_start(st_out[:], st_in[:])
    ast = sb.tile([D, 2], F32, tag="ast")
    nc.sync.dma_start(ast[:], st_out[:])

    # ---- BN coefficients (mlp_b cancels: z - mean(z) == v - mean(v)) ----
    bnt = sb.tile([D, 5], F32, tag="bnt")  # mu, ez2, sd, s, t
    nc.vector.tensor_scalar_mul(bnt[:, 0:1], ast[:, 0:1], 1.0 / B)
    nc.vector.tensor_scalar_mul(bnt[:, 1:2], ast[:, 1:2], 1.0 / B)
    nc.vector.tensor_mul(bnt[:, 2:3], bnt[:, 0:1], bnt[:, 0:1])
    nc.vector.tensor_sub(bnt[:, 1:2], bnt[:, 1:2], bnt[:, 2:3])
    nc.scalar.activation(bnt[:, 2:3], bnt[:, 1:2], AF.Sqrt, bias=epst[:, 0:1])
    nc.vector.reciprocal(bnt[:, 2:3], bnt[:, 2:3])
    nc.vector.tensor_mul(bnt[:, 3:4], gam, bnt[:, 2:3])
    nc.vector.tensor_mul(bnt[:, 4:5], bnt[:, 0:1], bnt[:, 3:4])
    nc.vector.tensor_sub(bnt[:, 4:5], bet, bnt[:, 4:5])

    # ---- zbn (bf16, per out group) interleaved with out tiles ----
    # out_i = zbn[:, tile_i]^T @ ehT; big copy half on DVE, small + zbn on Act
    zbn = sb.tile([D, BS], BF16, tag="zbn")
    og = outp.tile([128, BS // 128, N_USER], BF16, tag="og", bufs=1)
    for bi in range(BS // 128):
        if bi % 4 == 0:
            g = bi // 4
            nc.scalar.activation(zbn[:, g * QW:(g + 1) * QW],
                                 zt[:, g * QW:(g + 1) * QW], AF.Relu,
                                 bias=bnt[:, 4:5], scale=bnt[:, 3:4])
        oL = psT.tile([128, 450], F32, tag="oL", name=f"oL{bi}")
        oR = psT.tile([128, 355], F32, tag="oR", name=f"oR{bi}")
        lhs = zbn[:, 128 * bi:128 * (bi + 1)]
        nc.tensor.matmul(oL[:], lhs, eht[:, 0:450], start=True, stop=True)
        nc.tensor.matmul(oR[:], lhs, eht[:, 450:N_USER], start=True, stop=True)
        if bi % 2 == 0:
            nc.vector.tensor_copy(og[:, bi, 0:450], oL[:])
            nc.scalar.copy(og[:, bi, 450:N_USER], oR[:])
        else:
            nc.scalar.copy(og[:, bi, 0:450], oL[:])
            nc.vector.tensor_copy(og[:, bi, 450:N_USER], oR[:])
        if bi >= 12:
            nc.sync.dma_start(out[:, bi:bi + 1, :], og[:, bi:bi + 1, :])
        elif bi % 2 == 1:
            nc.sync.dma_start(out[:, bi - 1:bi + 1, :], og[:, bi - 1:bi + 1, :])


_NC_CACHE = {}


def _get_nc():
    if "nc" not in _NC_CACHE:
        _NC_CACHE["nc"] = _build()
    return _NC_CACHE["nc"]


def _prep(inputs):
    import ml_dtypes
    bf16 = ml_dtypes.bfloat16

    x_SH = np.asarray(inputs["x_SH"], dtype=np.int64)
    ei = np.asarray(inputs["edge_index_SH"])
    presc = np.asarray(inputs["prescription"], dtype=np.float32)
    SH_emb = np.asarray(inputs["SH_emb"], dtype=np.float32)
    W1 = np.asarray(inputs["W1"], dtype=np.float32)
    b1 = np.asarray(inputs["b1"], dtype=np.float32)
    W2 = np.asarray(inputs["W2"], dtype=np.float32)
    b2 = np.asarray(inputs["b2"], dtype=np.float32)
    mlp_W = np.asarray(inputs["mlp_W"], dtype=np.float32)
    gam = np.asarray(inputs["bn_gamma"], dtype=np.float32)
    bet = np.asarray(inputs["bn_beta"], dtype=np.float32)

    x1 = SH_emb[x_SH]                                       # (1195, 64)
    src = np.asarray(ei[0], dtype=np.int64)
    dst = np.asarray(ei[1], dtype=np.int64)
    stm = np.bincount(src * N_SH + dst, minlength=N_SH * N_SH).reshape(
        N_SH, N_SH).astype(np.float32)                      # S^T[s,d]
    cnt = stm.sum(axis=0)                                   # per-dst degree
    stnm = stm / np.maximum(cnt, 1.0)[None, :]              # normalized S^T

    def chunked(a, width):
        # (1195, w) -> zero-pad rows to 1280 -> (128, 10, w)
        p = np.zeros((NPAD, width), dtype=a.dtype)
        p[:N_SH] = a
        return np.ascontiguousarray(
            p.reshape(NKC, 128, width).transpose(1, 0, 2))

    stn_p = chunked(stnm.astype(bf16), N_SH)
    x1w_p = chunked((x1 @ W1.T).astype(bf16), D)

    nrm = np.sqrt((x1 * x1).sum(axis=1, keepdims=True))
    x1n = x1 / np.maximum(nrm, NORM_EPS)
    vec = np.stack([b1, b2, gam, bet], axis=1).astype(np.float32)
    par = np.concatenate([x1n.T, vec], axis=1)
    par = np.ascontiguousarray(par.astype(np.float32))
    assert par.shape == (D, PAR_W)
    wts = np.ascontiguousarray(
        np.concatenate([W2.T, mlp_W.T], axis=1).astype(bf16))

    shared = {"stn": stn_p, "x1w": x1w_p, "par": par, "wts": wts}
    in_maps = []
    for c in range(NCORES):
        xt = presc[c * BS:(c + 1) * BS].T.astype(bf16)      # (390, 2048)
        x012 = np.ascontiguousarray(
            xt[:384].reshape(3, 128, BS).transpose(1, 0, 2))
        m = dict(shared)
        m["xp"] = x012
        m["xp3"] = np.ascontiguousarray(xt[384:390])
        in_maps.append(m)
    return in_maps


def _assemble(res):
    outs = []
    for c in range(NCORES):
        o = np.asarray(res.results[c]["out"])               # (128, 16, 805) bf16
        outs.append(o.transpose(1, 0, 2).reshape(BS, N_USER))
    return np.concatenate(outs, axis=0).astype(np.float32)


def kernel(**inputs):
    in_maps = _prep(inputs)
    nc = _get_nc()
    res = run_bass_kernel_spmd(nc, in_maps, list(range(NCORES)))
    return _assemble(res)


def run_traced(inputs, tmpdir=None):
    """Profiled run: returns (output, exec_time_ns, results_obj)."""
    in_maps = _prep(inputs)
    nc = _get_nc()
    res = run_bass_kernel_spmd(nc, in_maps, list(range(NCORES)),
                               trace=True, tmpdir=tmpdir)
    return _assemble(res), res.exec_time_ns, res



# revision 60
# speedup vs baseline: 1.1470x; 1.1470x over previous
"""Trainium2 Bass kernel for nn_KDHR (gnn_message_passing), v2.

Math (per core, batch shard of 2048 rows):
  L1:  at = x1w^T @ StC   (StC = edge-count matrix S^T, exact small ints in
       fp8e4m3; x1w = SH_emb @ W1.T in bf16), then per-column scale by
       rdeg = 1/max(deg,1) on DVE, tanh(+b1) on Act -> h1t bf16.
  L2:  h1w_k = h1t chunk @ W2.T; bt = h1w^T @ StC; rdeg scale; tanh(+b2)
       -> cut (users, bf16), cit (items, bf16).
  es2 is never materialized: es2n_c = cit_c^T @ (rsqrt(colnorm) * mlp_W.T)
       + x1n_item@mlp_W.T (host), per 128-item chunk.
  zt quarters: esy = es2n^T @ X; one fused DVE op gives zt = esy*rp and
       the batch sum; squares accumulate on Act/Pool; BN stats [64,2]
       all-reduced (DRAM hops via Pool SWDGE to skip the shared HWDGE).
  PE warm-filler matmuls bridge the collective window (p-state stays hot).
  out tiles: zbn = relu(zt*s+t); per 128-row tile zbn^T @ ehT -> bf16.

The one AllReduce is replaced by a local DRAM copy when collective=False
(TimelineSim path used by test.py)."""

import os
import sys

for _p in ("/root/.axon_site", "/root/.axon_site/_ro/trn_rl_repo",
           "/root/.axon_site/_ro/pypackages", "/opt/trn_rl_repo", "/opt/pypackages"):
    if os.path.isdir(_p) and _p not in sys.path:
        sys.path.append(_p)

import numpy as np

import concourse.bass as bass
import concourse.mybir as mybir
import concourse.tile as tile
from concourse import bacc
from concourse.bass_utils import run_bass_kernel_spmd

N_USER, N_ITEM, N_SH, D = 805, 390, 1195, 64
B, NCORES = 16384, 8
BS = B // NCORES          # 2048 batch rows per core
NKC = 10                  # source-node chunks (1195 padded to 1280)
NPAD = NKC * 128
BN_EPS = 1e-5
NORM_EPS = 1e-12
F32 = mybir.dt.float32
BF16 = mybir.dt.bfloat16
F8 = mybir.dt.float8e4

STN_FP8 = True            # ship S^T as exact fp8 counts + device rdeg scale
DOUBLE_ROW = False         # fp8 DoubleRow aggs (hi/lo split keeps bf16 accuracy)
N_FILL_GROUPS = 6         # PE warm-filler groups of 6 matmuls in cc window

# agg col blocks, 805-aligned so the item block is a single PSUM tile
BLK_L1 = [("u0", 0, 512), ("u1", 512, 293), ("it", 805, 390)]
BLK_L2 = [("it", 805, 390), ("u0", 0, 512), ("u1", 512, 293)]
GR = ([(0, 2), (2, 2), (4, 2), (6, 2), (8, 2)] if DOUBLE_ROW else
      [(0, 1), (1, 2), (3, 2), (5, 2), (7, 2), (9, 1)])  # stn DMA groups
NQ = 4
QW = BS // NQ                                    # 512


def _build(collective=True):
    nc = bacc.Bacc("TRN2", target_bir_lowering=False, debug=False,
                   num_devices=NCORES)

    st_dt = F8 if STN_FP8 else BF16
    xp = nc.declare_dram_parameter("xp", [128, 3, BS], BF16, isOutput=False).ap()
    xp3 = nc.declare_dram_parameter("xp3", [6, BS], BF16, isOutput=False).ap()
    stn = nc.declare_dram_parameter("stn", [128, NKC, N_SH], st_dt, isOutput=False).ap()
    if DOUBLE_ROW:
        x1w = nc.declare_dram_parameter("x1w", [128, NKC, 2, D], F8,
                                        isOutput=False).ap()
    else:
        x1w = nc.declare_dram_parameter("x1w", [128, NKC, D], BF16,
                                        isOutput=False).ap()
    rdgb = nc.declare_dram_parameter("rdgb", [D, N_SH + 4], BF16, isOutput=False).ap()
    pv = nc.declare_dram_parameter("pv", [D, N_USER], BF16, isOutput=False).ap()
    xnm = nc.declare_dram_parameter("xnm", [128, 4, D], BF16, isOutput=False).ap()
    wts = nc.declare_dram_parameter("wts", [D, 2 * D], BF16, isOutput=False).ap()
    out = nc.declare_dram_parameter("out", [128, BS // 128, N_USER], BF16,
                                    isOutput=True).ap()

    from contextlib import ExitStack
    with tile.TileContext(nc) as tc, ExitStack() as ctx:
        pools = {
            "cst": ctx.enter_context(tc.tile_pool(name="cst", bufs=1)),
            "sb": ctx.enter_context(tc.tile_pool(name="sb", bufs=1)),
            "scr": ctx.enter_context(tc.tile_pool(name="scr", bufs=2)),
            "outp": ctx.enter_context(tc.tile_pool(name="outp", bufs=4)),
            "psA": ctx.enter_context(tc.tile_pool(name="psA", bufs=1, space="PSUM")),
            "psT": ctx.enter_context(tc.tile_pool(name="psT", bufs=2, space="PSUM")),
            "dram": ctx.enter_context(tc.tile_pool(name="dram", bufs=1, space="DRAM")),
        }
        _body(nc, tc, pools, xp, xp3, stn, x1w, rdgb, pv, xnm, wts,
              out, collective)

    nc.compile()
    return nc


def _body(nc, tc, P, xp, xp3, stn, x1w, rdgb, pv, xnm, wts, out,
          collective=True):
    AF = mybir.ActivationFunctionType
    ALU = mybir.AluOpType
    AX = mybir.AxisListType
    cst, sb, scr, outp = P["cst"], P["sb"], P["scr"], P["outp"]
    psA, psT, dram = P["psA"], P["psT"], P["dram"]
    st_dt = F8 if STN_FP8 else BF16

    # ---- constants + engine warm-up (no DMA deps) ----
    ones = cst.tile([128, D], BF16, tag="ones")
    nc.gpsimd.memset(ones[:], 1.0)
    epst = cst.tile([D, 1], F32, tag="epst")
    nc.vector.memset(epst[:], BN_EPS)
    warmact = cst.tile([D, 1], F32, tag="warmact")
    with tc.high_priority():
        nc.scalar.activation(warmact[:], epst[:], AF.Tanh)
    # PE warm touch: starts the p-state epoch early (borrows the oL slot)
    warmp = psT.tile([D, QW], F32, tag="oL", name="wp")
    nc.tensor.matmul(warmp[:, 0:D], ones[:], ones[:, :D], start=True, stop=True)

    # ---- DMAs, all via SP.  HWDGE descriptor-gen is a serialized shared
    # device and DMA transfers are serialized too, so issue order == need
    # order: stn chunk0 + x1w chunk0 gate L1's start. ----
    stg = []
    for gi, (g0, gn) in enumerate(GR):
        t = sb.tile([128, gn, N_SH], st_dt, tag=f"stn{gi}", name=f"stn{gi}")
        stg.append(t)
    if DOUBLE_ROW:
        x1w_sb = cst.tile([128, NKC, 2, D], F8, tag="x1w")
    else:
        x1w_sb = cst.tile([128, NKC, D], BF16, tag="x1w")
    k0n = 2 if DOUBLE_ROW else 1
    nc.sync.dma_start(stg[0][:], stn[:, 0:k0n, :])
    nc.sync.dma_start(x1w_sb[:, 0:k0n], x1w[:, 0:k0n])
    nc.sync.dma_start(x1w_sb[:, k0n:NKC], x1w[:, k0n:NKC])
    for gi, (g0, gn) in enumerate(GR[1:], start=1):
        nc.sync.dma_start(stg[gi][:], stn[:, g0:g0 + gn, :])

    wts_sb = cst.tile([D, 2 * D], BF16, tag="wts")
    nc.sync.dma_start(wts_sb[:], wts[:, :])
    w2b = wts_sb[:, 0:D]
    mwb = wts_sb[:, D:2 * D]
    pk_sb = cst.tile([D, N_SH + 4], BF16, tag="pk")  # rdeg rows | b1 b2 g b
    nc.sync.dma_start(pk_sb[:], rdgb[:, :])
    b1 = pk_sb[:, N_SH + 0:N_SH + 1]
    b2 = pk_sb[:, N_SH + 1:N_SH + 2]
    gam = pk_sb[:, N_SH + 2:N_SH + 3]
    bet = pk_sb[:, N_SH + 3:N_SH + 4]
    # X in 3 chunk-slices so the presum matmuls can start as each lands
    # (fills the PE gap between L1's end and the L2 agg)
    X = sb.tile([128, 3, BS], BF16, tag="X")
    nc.sync.dma_start(X[:, 0:1, :], xp[:, 0:1, :])
    nc.sync.dma_start(X[:, 1:2, :], xp[:, 1:2, :])
    X3 = sb.tile([6, BS], BF16, tag="X3")
    nc.sync.dma_start(X3[:], xp3[:, :])
    nc.sync.dma_start(X[:, 2:3, :], xp[:, 2:3, :])
    pv_sb = cst.tile([D, N_USER], BF16, tag="pv")      # x1nuT
    nc.sync.dma_start(pv_sb[:], pv[:, :])
    par_sb = pv_sb[:, 0:N_USER]
    xnm_sb = cst.tile([128, 4, D], BF16, tag="xnm")
    nc.sync.dma_start(xnm_sb[:], xnm[:, :, :])

    def st_chunk(k, c0, cn):
        for gi, (g0, gn) in enumerate(GR):
            if g0 <= k < g0 + gn:
                return stg[gi][:, k - g0, c0:c0 + cn]
        raise AssertionError(k)

    def st_pair(c, c0, cn):
        # [128, 2, cn] moving AP for DoubleRow chunk-pair c
        gi = c  # GR is [(0,2),(2,2),...] in DR mode
        return stg[gi][:, 0:2, c0:c0 + cn]

    # ---- L1: at = x1w^T @ StC, chunk-pipelined behind the stn DMAs.
    # Each col block accumulates in its OWN PSUM tile: readers on
    # different engines then never serialize at tile granularity. ----
    atb = {}
    for bn, c0, cn in BLK_L1:
        atb[bn] = psA.tile([D, cn], F32, tag=f"ag_{bn}", name=f"at_{bn}")
    if DOUBLE_ROW:
        DR = mybir.MatmulPerfMode.DoubleRow
        for c in range(NKC // 2):
            for hl in (0, 1):
                for bn, c0, cn in BLK_L1:
                    nc.tensor.matmul(atb[bn][:],
                                     x1w_sb[:, 2 * c:2 * c + 2, hl, :],
                                     st_pair(c, c0, cn), perf_mode=DR,
                                     start=(c == 0 and hl == 0),
                                     stop=(c == NKC // 2 - 1 and hl == 1))
    else:
        for k in range(NKC):
            for bn, c0, cn in BLK_L1:
                nc.tensor.matmul(atb[bn][:], x1w_sb[:, k, :], st_chunk(k, c0, cn),
                                 start=(k == 0), stop=(k == NKC - 1))
    h1t = sb.tile([D, NPAD], BF16, tag="h1t")
    nc.vector.memset(h1t[:, N_SH:NPAD], 0.0)
    if STN_FP8:
        atv = {bn: sb.tile([D, cn], F32, tag=f"atv_{bn}", name=f"atv_{bn}")
               for bn, c0, cn in BLK_L1}
        nc.vector.tensor_mul(atv["u0"][:], atb["u0"][:], pk_sb[:, 0:512])
        nc.vector.tensor_mul(atv["u1"][:], atb["u1"][:], pk_sb[:, 512:805])
        nc.vector.tensor_mul(atv["it"][:], atb["it"][:], pk_sb[:, 805:N_SH])
        for bn, c0, cn in BLK_L1:
            nc.scalar.activation(h1t[:, c0:c0 + cn], atv[bn][:], AF.Tanh,
                                 bias=b1)
    else:
        for bn, c0, cn in BLK_L1:
            nc.scalar.activation(h1t[:, c0:c0 + cn], atb[bn][:], AF.Tanh,
                                 bias=b1)

    # ---- L2 prep: h1w pairs (two matmuls into one PSUM bank, ONE copy op
    # per pair -> no tile-granular write/read interleaving stalls) ----
    h1w = []
    h1wl = []
    for p in range(NKC // 2):
        tp = psT.tile([128, 2, D], F32, tag="oL", name=f"h1wp{p}")
        for j in range(2):
            k = 2 * p + j
            nc.tensor.matmul(tp[:, j, :], h1t[:, 128 * k:128 * (k + 1)], w2b,
                             start=True, stop=True)
        if DOUBLE_ROW:
            hb = sb.tile([128, 2, D], F8, tag=f"h1w{p}", name=f"h1w{p}")
            nc.scalar.copy(hb[:], tp[:])
            lb = sb.tile([128, 2, D], F8, tag=f"h1wl{p}", name=f"h1wl{p}")
            nc.vector.scalar_tensor_tensor(lb[:], tp[:], 1.0, hb[:],
                                           ALU.bypass, ALU.subtract)
            h1wl.append(lb)
        else:
            hb = sb.tile([128, 2, D], BF16, tag=f"h1w{p}", name=f"h1w{p}")
            if p % 2 == 0:
                nc.vector.tensor_copy(hb[:], tp[:])
            else:
                nc.scalar.copy(hb[:], tp[:])
        h1w.append(hb)

    # ---- L2: bt = h1w^T @ StC (item block first: it gates the es chain) ----
    btb = {}
    for bn, c0, cn in BLK_L2:
        btb[bn] = psA.tile([D, cn], F32, tag=f"ag_{bn}", name=f"bt_{bn}")
    if DOUBLE_ROW:
        DR = mybir.MatmulPerfMode.DoubleRow
        for c in range(NKC // 2):
            for hl, hsrc in ((0, h1w), (1, h1wl)):
                for bn, c0, cn in BLK_L2:
                    nc.tensor.matmul(btb[bn][:], hsrc[c][:],
                                     st_pair(c, c0, cn), perf_mode=DR,
                                     start=(c == 0 and hl == 0),
                                     stop=(c == NKC // 2 - 1 and hl == 1))
    else:
        for k in range(NKC):
            for bn, c0, cn in BLK_L2:
                nc.tensor.matmul(btb[bn][:], h1w[k // 2][:, k % 2, :],
                                 st_chunk(k, c0, cn),
                                 start=(k == 0), stop=(k == NKC - 1))


    # ---- presum quarters (post-agg PE slot); recips split in halves so
    # the es-chain's DVE ops can slot between them ----
    rp_sb = sb.tile([D, BS], F32, tag="rp_sb")
    for q in range(NQ):
        t = psT.tile([D, QW], F32, tag="oR", name=f"ps{q}")
        nc.tensor.matmul(t[:], ones[:], X[:, 0, q * QW:(q + 1) * QW],
                         start=True, stop=False)
        nc.tensor.matmul(t[:], ones[:], X[:, 1, q * QW:(q + 1) * QW],
                         start=False, stop=False)
        nc.tensor.matmul(t[:], ones[:6, :], X3[:, q * QW:(q + 1) * QW],
                         start=False, stop=False)
        nc.tensor.matmul(t[:], ones[:], X[:, 2, q * QW:(q + 1) * QW],
                         start=False, stop=True)
        nc.vector.reciprocal(rp_sb[:, q * QW:(q + 1) * QW], t[:])

    # ---- item-side scale + tanh (gates the es chain) ----
    cit = sb.tile([D, 512], BF16, tag="cit")   # padded to 512 items
    nc.vector.memset(cit[:, N_ITEM:512], 0.0)
    cut = sb.tile([D, N_USER], BF16, tag="cut")
    rc = sb.tile([D, 4], F32, tag="rc")  # ssq_i, ssq_u, scale_i, scale_u
    sqj = scr.tile([D, 512], F32, tag="sqj")  # reduce junk
    if STN_FP8:
        btv_i = sb.tile([D, N_ITEM], F32, tag="btv_i")
        nc.vector.tensor_mul(btv_i[:], btb["it"][:], pk_sb[:, N_USER:N_SH])
        nc.scalar.activation(cit[:, 0:N_ITEM], btv_i[:], AF.Tanh, bias=b2)
    else:
        nc.scalar.activation(cit[:, 0:N_ITEM], btb["it"][:], AF.Tanh, bias=b2)

    # item col-norm on DVE: scale_i = sqrt(1/sumsq).  The Act table switch
    # (tanh -> sqrt set, 1283ns) rides on the rci Sqrt; the user tanh is
    # issued much later so it doesn't squeeze in front of the switch.
    nc.vector.scalar_tensor_tensor(
        sqj[:, 0:N_ITEM], cit[:, 0:N_ITEM], 1.0, cit[:, 0:N_ITEM],
        ALU.bypass, ALU.mult, accum_out=rc[:, 0:1])
    nc.vector.reciprocal(rc[:, 0:1], rc[:, 0:1])
    nc.scalar.activation(rc[:, 2:3], rc[:, 0:1], AF.Sqrt)
    mwbs = sb.tile([D, D], BF16, tag="mwbs")
    nc.scalar.activation(mwbs[:], mwb, AF.Copy, scale=rc[:, 2:3])
    # readiness gate: the user-side scale (and thus cut's tanh) must not
    # become schedulable before mwbs, or the Act queue runs it ahead of the
    # act-table switch and delays the es chain by ~2us
    gate1 = sb.tile([D, 1], F32, tag="gate1")
    nc.vector.tensor_scalar_mul(gate1[:], rc[:, 2:3], 0.0)
    nc.vector.tensor_scalar_add(gate1[:], gate1[:], 1.0)

    # ---- es2n pairs: es2n_c = cit_c^T @ mwbs + x1n_item@mlpW.T
    # (pair1 borrows the oR slot so the two pairs don't serialize on the
    # single tr bank; adds split DVE/Pool) ----
    es2n = []
    for p in range(2):
        tag = "tr" if p == 0 else "oR"
        bufs = {"bufs": 1} if p == 0 else {}
        tp = psT.tile([128, 2, D], F32, tag=tag, name=f"es2p{p}", **bufs)
        for j in range(2):
            c = 2 * p + j
            nc.tensor.matmul(tp[:, j, :], cit[:, 128 * c:128 * (c + 1)], mwbs[:],
                             start=True, stop=True)
        eb = sb.tile([128, 2, D], BF16, tag=f"es2n{p}", name=f"es2n{p}")
        nc.vector.tensor_add(eb[:], tp[:], xnm_sb[:, 2 * p:2 * p + 2, :])
        es2n.append(eb)


    # ---- esy quarters -> fused zt = esy*rp with batch-sum accum ----
    zt = sb.tile([D, BS], F32, tag="zt")
    s12 = sb.tile([D, 2 * NQ], F32, tag="s12")
    XC = [X[:, 0, :], X[:, 1, :], X[:, 2, :], X3]
    KN = [128, 128, 128, 6]
    for q in range(NQ):
        t = psT.tile([D, QW], F32, tag="oR", name=f"esy{q}")
        for c in range(4):
            eb = es2n[c // 2][:KN[c], c % 2, :]
            nc.tensor.matmul(t[:], eb, XC[c][:KN[c], q * QW:(q + 1) * QW],
                             start=(c == 0), stop=(c == 3))
        ztq = zt[:, q * QW:(q + 1) * QW]
        nc.vector.scalar_tensor_tensor(
            ztq, t[:], 1.0, rp_sb[:, q * QW:(q + 1) * QW],
            ALU.bypass, ALU.mult, accum_out=s12[:, q:q + 1])
        if q < 2:
            sq = scr.tile([D, QW], F32, tag="sq")
            nc.scalar.activation(sq[:], ztq, AF.Square,
                                 accum_out=s12[:, NQ + q:NQ + q + 1])
        else:
            sq3 = scr.tile([D, QW], F32, tag="sq")
            nc.vector.scalar_tensor_tensor(
                sq3[:], ztq, 1.0, ztq, ALU.bypass, ALU.mult,
                accum_out=s12[:, NQ + q:NQ + q + 1])

    stats = sb.tile([D, 2], F32, tag="stats")
    nc.vector.tensor_reduce(stats[:, 0:1], s12[:, 0:NQ], axis=AX.X, op=ALU.add)
    nc.vector.tensor_reduce(stats[:, 1:2], s12[:, NQ:2 * NQ], axis=AX.X, op=ALU.add)

    # user-side tanh + norm + ehT: gated behind the es chain (gate1)
    btc_u = sb.tile([D, N_USER], F32, tag="btc_u")
    nc.scalar.activation(btc_u[:, 0:512], btb["u0"][:], AF.Copy,
                         scale=gate1[:, 0:1])
    nc.scalar.activation(btc_u[:, 512:N_USER], btb["u1"][:], AF.Copy,
                         scale=gate1[:, 0:1])
    if STN_FP8:
        btv_u = sb.tile([D, N_USER], F32, tag="btv_u")
        nc.gpsimd.tensor_mul(btv_u[:], btc_u[:], pk_sb[:, 0:N_USER])
        cut_in = btv_u[:]
    else:
        cut_in = btc_u[:]
    nc.scalar.activation(cut[:], cut_in, AF.Tanh, bias=b2)
    sqc = scr.tile([D, N_USER], F32, tag="sqc")
    nc.scalar.activation(sqc[:], cut[:], AF.Square, accum_out=rc[:, 1:2])
    nc.vector.reciprocal(rc[:, 1:2], rc[:, 1:2])
    rcu = rc[:, 3:4]
    nc.scalar.activation(rcu, rc[:, 1:2], AF.Sqrt)
    ecu = sb.tile([D, N_USER], F32, tag="ecu")
    nc.scalar.activation(ecu[:], cut[:], AF.Copy, scale=rcu)
    eht = sb.tile([D, N_USER], BF16, tag="eht")
    nc.gpsimd.tensor_add(eht[:], ecu[:], par_sb)

    # ---- all-reduce BN stats ([64,2]) ----
    st_in = dram.tile([D, 2], F32, tag="cc_in")
    st_out = dram.tile([D, 2], F32, tag="cc_out")
    nc.sync.dma_start(st_in[:], stats[:])

    if collective:
        nc.gpsimd.collective_compute(
            "AllReduce", mybir.AluOpType.add,
            replica_groups=[list(range(NCORES))],
            ins=[st_in.opt()], outs=[st_out.opt()])
    else:
        nc.sync.dma_start(st_out[:], st_in[:])
    ast = sb.tile([D, 2], F32, tag="ast")
    nc.sync.dma_start(ast[:], st_out[:])

    # ---- PE warm-filler bridges the collective window.  The moving
    # tensor is `cut` (ready only after the es chain) so fillers cannot
    # preempt the es2n/esy matmuls. ----
    for g in range(N_FILL_GROUPS):
        warm = psT.tile([D, QW], F32, tag="oL", name=f"warm{g}")
        for w in range(6):
            nc.tensor.matmul(warm[:], ones[:D, :], cut[:, 0:QW],
                             start=(w == 0), stop=(w == 5))

    # ---- BN coefficients (mlp_b cancels inside BN) ----
    bnt = sb.tile([D, 5], F32, tag="bnt")  # mu, var, mu2, s, t
    nc.vector.tensor_scalar_mul(bnt[:, 0:1], ast[:, 0:1], 1.0 / B)
    nc.vector.tensor_scalar_mul(bnt[:, 1:2], ast[:, 1:2], 1.0 / B)
    nc.vector.tensor_mul(bnt[:, 2:3], bnt[:, 0:1], bnt[:, 0:1])
    nc.vector.tensor_sub(bnt[:, 1:2], bnt[:, 1:2], bnt[:, 2:3])
    nc.scalar.activation(bnt[:, 2:3], bnt[:, 1:2], AF.Sqrt, bias=epst[:, 0:1])
    nc.vector.reciprocal(bnt[:, 2:3], bnt[:, 2:3])
    nc.vector.tensor_mul(bnt[:, 3:4], gam, bnt[:, 2:3])
    nc.vector.tensor_mul(bnt[:, 4:5], bnt[:, 0:1], bnt[:, 3:4])
    nc.vector.tensor_sub(bnt[:, 4:5], bet, bnt[:, 4:5])

    # ---- zbn (bf16, per out group) interleaved with out tiles;
    # PSUM->SBUF copies split 3 ways (DVE / Act / Pool) ----
    zbn = sb.tile([D, BS], BF16, tag="zbn")
    og = outp.tile([128, BS // 128, N_USER], BF16, tag="og", bufs=1)
    NT = BS // 128
    for bi in range(NT):
        if bi % 4 == 0:
            g = bi // 4
            nc.scalar.activation(zbn[:, g * QW:(g + 1) * QW],
                                 zt[:, g * QW:(g + 1) * QW], AF.Relu,
                                 bias=bnt[:, 4:5], scale=bnt[:, 3:4])
        oL = psT.tile([128, 450], F32, tag="oL", name=f"oL{bi}")
        oR = psT.tile([128, 355], F32, tag="oR", name=f"oR{bi}")
        lhs = zbn[:, 128 * bi:128 * (bi + 1)]
        nc.tensor.matmul(oL[:], lhs, eht[:, 0:450], start=True, stop=True)
        nc.tensor.matmul(oR[:], lhs, eht[:, 450:N_USER], start=True, stop=True)
        if bi % 2 == 0:
            nc.vector.tensor_copy(og[:, bi, 0:450], oL[:])
            nc.scalar.copy(og[:, bi, 450:N_USER], oR[:])
        else:
            nc.scalar.copy(og[:, bi, 0:450], oL[:])
            nc.vector.tensor_copy(og[:, bi, 450:N_USER], oR[:])
        if bi == 0 or bi == NT - 1:
            nc.sync.dma_start(out[:, bi:bi + 1, :], og[:, bi:bi + 1, :])
        elif bi % 2 == 0:
            nc.sync.dma_start(out[:, bi - 1:bi + 1, :], og[:, bi - 1:bi + 1, :])


_NC_CACHE = {}


def _get_nc():
    if "nc" not in _NC_CACHE:
        _NC_CACHE["nc"] = _build()
    return _NC_CACHE["nc"]


def _prep(inputs):
    import ml_dtypes
    bf16 = ml_dtypes.bfloat16
    f8 = ml_dtypes.float8_e4m3

    x_SH = np.asarray(inputs["x_SH"], dtype=np.int64)
    ei = np.asarray(inputs["edge_index_SH"])
    presc = np.asarray(inputs["prescription"], dtype=np.float32)
    SH_emb = np.asarray(inputs["SH_emb"], dtype=np.float32)
    W1 = np.asarray(inputs["W1"], dtype=np.float32)
    b1 = np.asarray(inputs["b1"], dtype=np.float32)
    W2 = np.asarray(inputs["W2"], dtype=np.float32)
    b2 = np.asarray(inputs["b2"], dtype=np.float32)
    mlp_W = np.asarray(inputs["mlp_W"], dtype=np.float32)
    gam = np.asarray(inputs["bn_gamma"], dtype=np.float32)
    bet = np.asarray(inputs["bn_beta"], dtype=np.float32)

    x1 = SH_emb[x_SH]                                       # (1195, 64)
    src = np.asarray(ei[0], dtype=np.int64)
    dst = np.asarray(ei[1], dtype=np.int64)
    stm = np.bincount(src * N_SH + dst, minlength=N_SH * N_SH).reshape(
        N_SH, N_SH).astype(np.float32)                      # S^T[s,d] counts
    cnt = stm.sum(axis=0)                                   # per-dst degree
    rdeg = 1.0 / np.maximum(cnt, 1.0)                       # (1195,)

    def chunked(a, width):
        # (1195, w) -> zero-pad rows to 1280 -> (128, 10, w)
        p = np.zeros((NPAD, width), dtype=a.dtype)
        p[:N_SH] = a
        return np.ascontiguousarray(
            p.reshape(NKC, 128, width).transpose(1, 0, 2))

    if STN_FP8:
        assert stm.max() <= 15, stm.max()     # fp8e4m3 integers exact to 16
        stn_p = chunked(stm.astype(f8), N_SH)
    else:
        stn_p = chunked((stm * rdeg[None, :]).astype(bf16), N_SH)
    x1w_full = x1 @ W1.T
    if DOUBLE_ROW:
        x1w_hi = x1w_full.astype(f8)
        x1w_lo = (x1w_full - x1w_hi.astype(np.float32)).astype(f8)
        hilo = np.stack([x1w_hi, x1w_lo], axis=1)        # (1195, 2, 64)
        x1w_p = chunked(hilo.reshape(N_SH, 2 * D), 2 * D).reshape(
            128, NKC, 2, D)
    else:
        x1w_p = chunked(x1w_full.astype(bf16), D)
    rdg_p = np.broadcast_to(rdeg[None, :], (D, N_SH)).astype(np.float32)

    nrm = np.sqrt((x1 * x1).sum(axis=1, keepdims=True))
    x1n = x1 / np.maximum(nrm, NORM_EPS)                    # (1195, 64)
    vec = np.stack([b1, b2, gam, bet], axis=1).astype(np.float32)
    par = x1n[:N_USER].T.astype(np.float32)                        # (64, 805)
    rdgb = np.ascontiguousarray(
        np.concatenate([rdg_p, vec], axis=1).astype(bf16))
    pv = np.ascontiguousarray(par.astype(bf16))
    xnm_full = x1n[N_USER:] @ mlp_W.T                       # (390, 64)
    xnm_pad = np.zeros((512, D), np.float32)
    xnm_pad[:N_ITEM] = xnm_full
    xnm_p = np.ascontiguousarray(
        xnm_pad.reshape(4, 128, D).transpose(1, 0, 2).astype(bf16))
    wts = np.ascontiguousarray(
        np.concatenate([W2.T, mlp_W.T], axis=1).astype(bf16))

    shared = {"stn": stn_p, "x1w": x1w_p, "rdgb": rdgb, "pv": pv,
              "xnm": xnm_p, "wts": wts}

    in_maps = []
    for c in range(NCORES):
        xt = presc[c * BS:(c + 1) * BS].T.astype(bf16)      # (390, 2048)
        x012 = np.ascontiguousarray(
            xt[:384].reshape(3, 128, BS).transpose(1, 0, 2))
        m = dict(shared)
        m["xp"] = x012
        m["xp3"] = np.ascontiguousarray(xt[384:390])
        in_maps.append(m)
    return in_maps


def _assemble(res):
    outs = []
    for c in range(NCORES):
        o = np.asarray(res.results[c]["out"])               # (128, 16, 805) bf16
        outs.append(o.transpose(1, 0, 2).reshape(BS, N_USER))
    return np.concatenate(outs, axis=0).astype(np.float32)


def kernel(**inputs):
    in_maps = _prep(inputs)
    nc = _get_nc()
    res = run_bass_kernel_spmd(nc, in_maps, list(range(NCORES)))
    return _assemble(res)


def run_traced(inputs, tmpdir=None):
    """Profiled run: returns (output, exec_time_ns, results_obj)."""
    in_maps = _prep(inputs)
    nc = _get_nc()
    res = run_bass_kernel_spmd(nc, in_maps, list(range(NCORES)),
                               trace=True, tmpdir=tmpdir)
    return _assemble(res), res.exec_time_ns, res


# revision 63
# speedup vs baseline: 1.1480x; 1.0009x over previous
"""Trainium2 Bass kernel for nn_KDHR (gnn_message_passing), v2.

Batch rows are sharded 2048/core across 8 cores; the tiny graph state is
replicated.  Per core:

  L1:  at = x1w^T @ StC.  StC is the edge-count matrix S^T shipped as
       EXACT small integers in fp8e4m3 (halves the dominant DMA); the
       1/deg column scale is applied on DVE afterwards, tanh(+b1) on Act.
       Each 805-aligned column block accumulates in its own PSUM tile so
       cross-engine readers never serialize at tile granularity.
  L2:  h1w pairs (two 128-chunks per PSUM bank, one copy per pair);
       bt = h1w^T @ StC with the ITEM block first - it gates the es chain.
  es:  es2 is never materialized: es2n_c = cit_c^T @ (sqrt(1/ssq)*mlpW^T)
       + x1n_item@mlpW^T (host-precomputed).  The single act-table switch
       (tanh -> sqrt set, 1283ns) fires right after the item tanh and
       overlaps the DVE column-norm reduce.  The user-side tanh/norm/ehT
       chain is readiness-gated (gate1) behind the es chain and fills
       Act/Pool slack during the esy era and the collective window.
  zt:  esy = es2n^T @ X per batch quarter; ONE fused DVE op forms
       zt = esy*(1/presum) AND the batch sum; sums of squares split
       Act (q0/q1) / DVE (q2/q3).  Presum quarters ride the oL PSUM
       rotation right after the L2 agg; recips chase on DVE.
  BN:  stats [64,2] all-reduced over 8 cores (DRAM staging hops + one
       AllReduce); PE warm-filler matmuls (moving tensor = cut, so they
       can never preempt real work) bridge the window and keep the
       tensor engine's p-state at full clock for the out stage.
  out: zbn = relu(zt*s+t) per quarter; per 128-row tile zbn^T @ ehT,
       PSUM->SBUF copies split DVE/Act, paired DMAs (single first/last)
       stream bf16 tiles out; the host concatenates and casts.

The one AllReduce is replaced by a local DRAM copy when collective=False
(the TimelineSim path test.py measures)."""

import os
import sys

for _p in ("/root/.axon_site", "/root/.axon_site/_ro/trn_rl_repo",
           "/root/.axon_site/_ro/pypackages", "/opt/trn_rl_repo", "/opt/pypackages"):
    if os.path.isdir(_p) and _p not in sys.path:
        sys.path.append(_p)

import numpy as np

import concourse.bass as bass
import concourse.mybir as mybir
import concourse.tile as tile
from concourse import bacc
from concourse.bass_utils import run_bass_kernel_spmd

N_USER, N_ITEM, N_SH, D = 805, 390, 1195, 64
B, NCORES = 16384, 8
BS = B // NCORES          # 2048 batch rows per core
NKC = 10                  # source-node chunks (1195 padded to 1280)
NPAD = NKC * 128
BN_EPS = 1e-5
NORM_EPS = 1e-12
F32 = mybir.dt.float32
BF16 = mybir.dt.bfloat16
F8 = mybir.dt.float8e4

STN_FP8 = True            # ship S^T as exact fp8 counts + device rdeg scale
DOUBLE_ROW = False         # fp8 DoubleRow aggs (hi/lo split keeps bf16 accuracy)
N_FILL_GROUPS = 6         # PE warm-filler groups of 6 matmuls in cc window

# agg col blocks, 805-aligned so the item block is a single PSUM tile
BLK_L1 = [("u0", 0, 512), ("u1", 512, 293), ("it", 805, 390)]
BLK_L2 = [("it", 805, 390), ("u0", 0, 512), ("u1", 512, 293)]
GR = ([(0, 2), (2, 2), (4, 2), (6, 2), (8, 2)] if DOUBLE_ROW else
      [(0, 1), (1, 2), (3, 2), (5, 2), (7, 2), (9, 1)])  # stn DMA groups
NQ = 4
QW = BS // NQ                                    # 512


def _build(collective=True):
    nc = bacc.Bacc("TRN2", target_bir_lowering=False, debug=False,
                   num_devices=NCORES)

    st_dt = F8 if STN_FP8 else BF16
    xp = nc.declare_dram_parameter("xp", [128, 3, BS], BF16, isOutput=False).ap()
    xp3 = nc.declare_dram_parameter("xp3", [6, BS], BF16, isOutput=False).ap()
    stn = nc.declare_dram_parameter("stn", [128, NKC, N_SH], st_dt, isOutput=False).ap()
    if DOUBLE_ROW:
        x1w = nc.declare_dram_parameter("x1w", [128, NKC, 2, D], F8,
                                        isOutput=False).ap()
    else:
        x1w = nc.declare_dram_parameter("x1w", [128, NKC, D], BF16,
                                        isOutput=False).ap()
    rdgb = nc.declare_dram_parameter("rdgb", [D, N_SH + 4], BF16, isOutput=False).ap()
    pv = nc.declare_dram_parameter("pv", [D, N_USER], BF16, isOutput=False).ap()
    xnm = nc.declare_dram_parameter("xnm", [128, 4, D], BF16, isOutput=False).ap()
    wts = nc.declare_dram_parameter("wts", [D, 2 * D], BF16, isOutput=False).ap()
    out = nc.declare_dram_parameter("out", [128, BS // 128, N_USER], BF16,
                                    isOutput=True).ap()

    from contextlib import ExitStack
    with tile.TileContext(nc) as tc, ExitStack() as ctx:
        pools = {
            "cst": ctx.enter_context(tc.tile_pool(name="cst", bufs=1)),
            "sb": ctx.enter_context(tc.tile_pool(name="sb", bufs=1)),
            "scr": ctx.enter_context(tc.tile_pool(name="scr", bufs=2)),
            "outp": ctx.enter_context(tc.tile_pool(name="outp", bufs=4)),
            "psA": ctx.enter_context(tc.tile_pool(name="psA", bufs=1, space="PSUM")),
            "psT": ctx.enter_context(tc.tile_pool(name="psT", bufs=2, space="PSUM")),
            "dram": ctx.enter_context(tc.tile_pool(name="dram", bufs=1, space="DRAM")),
        }
        _body(nc, tc, pools, xp, xp3, stn, x1w, rdgb, pv, xnm, wts,
              out, collective)

    nc.compile()
    return nc


def _body(nc, tc, P, xp, xp3, stn, x1w, rdgb, pv, xnm, wts, out,
          collective=True):
    AF = mybir.ActivationFunctionType
    ALU = mybir.AluOpType
    AX = mybir.AxisListType
    cst, sb, scr, outp = P["cst"], P["sb"], P["scr"], P["outp"]
    psA, psT, dram = P["psA"], P["psT"], P["dram"]
    st_dt = F8 if STN_FP8 else BF16

    # ---- constants + engine warm-up (no DMA deps) ----
    ones = cst.tile([128, D], BF16, tag="ones")
    nc.gpsimd.memset(ones[:], 1.0)
    epst = cst.tile([D, 1], F32, tag="epst")
    nc.vector.memset(epst[:], BN_EPS)
    warmact = cst.tile([D, 1], F32, tag="warmact")
    with tc.high_priority():
        nc.scalar.activation(warmact[:], epst[:], AF.Tanh)
    # PE warm touch: starts the p-state epoch early (borrows the oL slot)
    warmp = psT.tile([D, QW], F32, tag="oL", name="wp")
    nc.tensor.matmul(warmp[:, 0:D], ones[:], ones[:, :D], start=True, stop=True)

    # ---- DMAs, all via SP.  HWDGE descriptor-gen is a serialized shared
    # device and DMA transfers are serialized too, so issue order == need
    # order: stn chunk0 + x1w chunk0 gate L1's start. ----
    stg = []
    for gi, (g0, gn) in enumerate(GR):
        t = sb.tile([128, gn, N_SH], st_dt, tag=f"stn{gi}", name=f"stn{gi}")
        stg.append(t)
    if DOUBLE_ROW:
        x1w_sb = cst.tile([128, NKC, 2, D], F8, tag="x1w")
    else:
        x1w_sb = cst.tile([128, NKC, D], BF16, tag="x1w")
    k0n = 2 if DOUBLE_ROW else 1
    nc.sync.dma_start(stg[0][:], stn[:, 0:k0n, :])
    nc.sync.dma_start(x1w_sb[:, 0:k0n], x1w[:, 0:k0n])
    nc.sync.dma_start(x1w_sb[:, k0n:NKC], x1w[:, k0n:NKC])
    for gi, (g0, gn) in enumerate(GR[1:], start=1):
        nc.sync.dma_start(stg[gi][:], stn[:, g0:g0 + gn, :])

    wts_sb = cst.tile([D, 2 * D], BF16, tag="wts")
    nc.sync.dma_start(wts_sb[:], wts[:, :])
    w2b = wts_sb[:, 0:D]
    mwb = wts_sb[:, D:2 * D]
    pk_sb = cst.tile([D, N_SH + 4], BF16, tag="pk")  # rdeg rows | b1 b2 g b
    nc.sync.dma_start(pk_sb[:], rdgb[:, :])
    b1 = pk_sb[:, N_SH + 0:N_SH + 1]
    b2 = pk_sb[:, N_SH + 1:N_SH + 2]
    gam = pk_sb[:, N_SH + 2:N_SH + 3]
    bet = pk_sb[:, N_SH + 3:N_SH + 4]
    # X in 3 chunk-slices so the presum matmuls can start as each lands
    # (fills the PE gap between L1's end and the L2 agg)
    X = sb.tile([128, 3, BS], BF16, tag="X")
    nc.sync.dma_start(X[:, 0:1, :], xp[:, 0:1, :])
    nc.sync.dma_start(X[:, 1:2, :], xp[:, 1:2, :])
    X3 = sb.tile([6, BS], BF16, tag="X3")
    nc.sync.dma_start(X3[:], xp3[:, :])
    nc.sync.dma_start(X[:, 2:3, :], xp[:, 2:3, :])
    pv_sb = cst.tile([D, N_USER], BF16, tag="pv")      # x1nuT
    nc.sync.dma_start(pv_sb[:], pv[:, :])
    par_sb = pv_sb[:, 0:N_USER]
    xnm_sb = cst.tile([128, 4, D], BF16, tag="xnm")
    nc.sync.dma_start(xnm_sb[:], xnm[:, :, :])

    def st_chunk(k, c0, cn):
        for gi, (g0, gn) in enumerate(GR):
            if g0 <= k < g0 + gn:
                return stg[gi][:, k - g0, c0:c0 + cn]
        raise AssertionError(k)

    def st_pair(c, c0, cn):
        # [128, 2, cn] moving AP for DoubleRow chunk-pair c
        gi = c  # GR is [(0,2),(2,2),...] in DR mode
        return stg[gi][:, 0:2, c0:c0 + cn]

    # ---- L1: at = x1w^T @ StC, chunk-pipelined behind the stn DMAs.
    # Each col block accumulates in its OWN PSUM tile: readers on
    # different engines then never serialize at tile granularity. ----
    atb = {}
    for bn, c0, cn in BLK_L1:
        atb[bn] = psA.tile([D, cn], F32, tag=f"ag_{bn}", name=f"at_{bn}")
    if DOUBLE_ROW:
        DR = mybir.MatmulPerfMode.DoubleRow
        for c in range(NKC // 2):
            for hl in (0, 1):
                for bn, c0, cn in BLK_L1:
                    nc.tensor.matmul(atb[bn][:],
                                     x1w_sb[:, 2 * c:2 * c + 2, hl, :],
                                     st_pair(c, c0, cn), perf_mode=DR,
                                     start=(c == 0 and hl == 0),
                                     stop=(c == NKC // 2 - 1 and hl == 1))
    else:
        for k in range(NKC):
            for bn, c0, cn in BLK_L1:
                nc.tensor.matmul(atb[bn][:], x1w_sb[:, k, :], st_chunk(k, c0, cn),
                                 start=(k == 0), stop=(k == NKC - 1))
    h1t = sb.tile([D, NPAD], BF16, tag="h1t")
    nc.vector.memset(h1t[:, N_SH:NPAD], 0.0)
    if STN_FP8:
        atv = {bn: sb.tile([D, cn], F32, tag=f"atv_{bn}", name=f"atv_{bn}")
               for bn, c0, cn in BLK_L1}
        nc.vector.tensor_mul(atv["u0"][:], atb["u0"][:], pk_sb[:, 0:512])
        nc.vector.tensor_mul(atv["u1"][:], atb["u1"][:], pk_sb[:, 512:805])
        nc.vector.tensor_mul(atv["it"][:], atb["it"][:], pk_sb[:, 805:N_SH])
        for bn, c0, cn in BLK_L1:
            nc.scalar.activation(h1t[:, c0:c0 + cn], atv[bn][:], AF.Tanh,
                                 bias=b1)
    else:
        for bn, c0, cn in BLK_L1:
            nc.scalar.activation(h1t[:, c0:c0 + cn], atb[bn][:], AF.Tanh,
                                 bias=b1)

    # ---- L2 prep: h1w pairs (two matmuls into one PSUM bank, ONE copy op
    # per pair -> no tile-granular write/read interleaving stalls) ----
    h1w = []
    h1wl = []
    for p in range(NKC // 2):
        tp = psT.tile([128, 2, D], F32, tag="oL", name=f"h1wp{p}")
        for j in range(2):
            k = 2 * p + j
            nc.tensor.matmul(tp[:, j, :], h1t[:, 128 * k:128 * (k + 1)], w2b,
                             start=True, stop=True)
        if DOUBLE_ROW:
            hb = sb.tile([128, 2, D], F8, tag=f"h1w{p}", name=f"h1w{p}")
            nc.scalar.copy(hb[:], tp[:])
            lb = sb.tile([128, 2, D], F8, tag=f"h1wl{p}", name=f"h1wl{p}")
            nc.vector.scalar_tensor_tensor(lb[:], tp[:], 1.0, hb[:],
                                           ALU.bypass, ALU.subtract)
            h1wl.append(lb)
        else:
            hb = sb.tile([128, 2, D], BF16, tag=f"h1w{p}", name=f"h1w{p}")
            if p % 2 == 0:
                nc.vector.tensor_copy(hb[:], tp[:])
            else:
                nc.scalar.copy(hb[:], tp[:])
        h1w.append(hb)

    # ---- L2: bt = h1w^T @ StC (item block first: it gates the es chain) ----
    btb = {}
    for bn, c0, cn in BLK_L2:
        btb[bn] = psA.tile([D, cn], F32, tag=f"ag_{bn}", name=f"bt_{bn}")
    if DOUBLE_ROW:
        DR = mybir.MatmulPerfMode.DoubleRow
        for c in range(NKC // 2):
            for hl, hsrc in ((0, h1w), (1, h1wl)):
                for bn, c0, cn in BLK_L2:
                    nc.tensor.matmul(btb[bn][:], hsrc[c][:],
                                     st_pair(c, c0, cn), perf_mode=DR,
                                     start=(c == 0 and hl == 0),
                                     stop=(c == NKC // 2 - 1 and hl == 1))
    else:
        for k in range(NKC):
            for bn, c0, cn in BLK_L2:
                nc.tensor.matmul(btb[bn][:], h1w[k // 2][:, k % 2, :],
                                 st_chunk(k, c0, cn),
                                 start=(k == 0), stop=(k == NKC - 1))


    # ---- presum quarters (post-agg PE slot); recips split in halves so
    # the es-chain's DVE ops can slot between them ----
    rp_sb = sb.tile([D, BS], F32, tag="rp_sb")
    for q in range(NQ):
        t = psT.tile([D, QW], F32, tag="oR", name=f"ps{q}")
        nc.tensor.matmul(t[:], ones[:], X[:, 0, q * QW:(q + 1) * QW],
                         start=True, stop=False)
        nc.tensor.matmul(t[:], ones[:], X[:, 1, q * QW:(q + 1) * QW],
                         start=False, stop=False)
        nc.tensor.matmul(t[:], ones[:6, :], X3[:, q * QW:(q + 1) * QW],
                         start=False, stop=False)
        nc.tensor.matmul(t[:], ones[:], X[:, 2, q * QW:(q + 1) * QW],
                         start=False, stop=True)
        nc.vector.reciprocal(rp_sb[:, q * QW:(q + 1) * QW], t[:])

    # ---- item-side scale + tanh (gates the es chain) ----
    cit = sb.tile([D, 512], BF16, tag="cit")   # padded to 512 items
    nc.vector.memset(cit[:, N_ITEM:512], 0.0)
    cut = sb.tile([D, N_USER], BF16, tag="cut")
    rc = sb.tile([D, 4], F32, tag="rc")  # ssq_i, ssq_u, scale_i, scale_u
    sqj = scr.tile([D, 512], F32, tag="sqj")  # reduce junk
    if STN_FP8:
        btv_i = sb.tile([D, N_ITEM], F32, tag="btv_i")
        nc.vector.tensor_mul(btv_i[:], btb["it"][:], pk_sb[:, N_USER:N_SH])
        nc.scalar.activation(cit[:, 0:N_ITEM], btv_i[:], AF.Tanh, bias=b2)
    else:
        nc.scalar.activation(cit[:, 0:N_ITEM], btb["it"][:], AF.Tanh, bias=b2)

    # item col-norm on DVE: scale_i = sqrt(1/sumsq).  The Act table switch
    # (tanh -> sqrt set, 1283ns) rides on the rci Sqrt; the user tanh is
    # issued much later so it doesn't squeeze in front of the switch.
    nc.vector.scalar_tensor_tensor(
        sqj[:, 0:N_ITEM], cit[:, 0:N_ITEM], 1.0, cit[:, 0:N_ITEM],
        ALU.bypass, ALU.mult, accum_out=rc[:, 0:1])
    nc.vector.reciprocal(rc[:, 0:1], rc[:, 0:1])
    nc.scalar.activation(rc[:, 2:3], rc[:, 0:1], AF.Sqrt)
    mwbs = sb.tile([D, D], BF16, tag="mwbs")
    nc.scalar.activation(mwbs[:], mwb, AF.Copy, scale=rc[:, 2:3])
    # readiness gate: the user-side scale (and thus cut's tanh) must not
    # become schedulable before mwbs, or the Act queue runs it ahead of the
    # act-table switch and delays the es chain by ~2us
    gate1 = sb.tile([D, 1], F32, tag="gate1")
    nc.vector.tensor_scalar_mul(gate1[:], rc[:, 2:3], 0.0)
    nc.vector.tensor_scalar_add(gate1[:], gate1[:], 1.0)

    # ---- es2n pairs: es2n_c = cit_c^T @ mwbs + x1n_item@mlpW.T
    # (pair1 borrows the oR slot so the two pairs don't serialize on the
    # single tr bank; adds split DVE/Pool) ----
    es2n = []
    for p in range(2):
        tag = "tr" if p == 0 else "oR"
        bufs = {"bufs": 1} if p == 0 else {}
        tp = psT.tile([128, 2, D], F32, tag=tag, name=f"es2p{p}", **bufs)
        for j in range(2):
            c = 2 * p + j
            nc.tensor.matmul(tp[:, j, :], cit[:, 128 * c:128 * (c + 1)], mwbs[:],
                             start=True, stop=True)
        eb = sb.tile([128, 2, D], BF16, tag=f"es2n{p}", name=f"es2n{p}")
        nc.vector.tensor_add(eb[:], tp[:], xnm_sb[:, 2 * p:2 * p + 2, :])
        es2n.append(eb)


    # ---- esy quarters -> fused zt = esy*rp with batch-sum accum ----
    zt = sb.tile([D, BS], F32, tag="zt")
    s12 = sb.tile([D, 2 * NQ], F32, tag="s12")
    XC = [X[:, 0, :], X[:, 1, :], X[:, 2, :], X3]
    KN = [128, 128, 128, 6]
    for q in range(NQ):
        t = psT.tile([D, QW], F32, tag="oR", name=f"esy{q}")
        for c in range(4):
            eb = es2n[c // 2][:KN[c], c % 2, :]
            nc.tensor.matmul(t[:], eb, XC[c][:KN[c], q * QW:(q + 1) * QW],
                             start=(c == 0), stop=(c == 3))
        ztq = zt[:, q * QW:(q + 1) * QW]
        nc.vector.scalar_tensor_tensor(
            ztq, t[:], 1.0, rp_sb[:, q * QW:(q + 1) * QW],
            ALU.bypass, ALU.mult, accum_out=s12[:, q:q + 1])
        if q < 2:
            sq = scr.tile([D, QW], F32, tag="sq")
            nc.scalar.activation(sq[:], ztq, AF.Square,
                                 accum_out=s12[:, NQ + q:NQ + q + 1])
        else:
            sq3 = scr.tile([D, QW], F32, tag="sq")
            nc.vector.scalar_tensor_tensor(
                sq3[:], ztq, 1.0, ztq, ALU.bypass, ALU.mult,
                accum_out=s12[:, NQ + q:NQ + q + 1])

    stats = sb.tile([D, 2], F32, tag="stats")
    nc.vector.tensor_reduce(stats[:, 0:1], s12[:, 0:NQ], axis=AX.X, op=ALU.add)
    nc.vector.tensor_reduce(stats[:, 1:2], s12[:, NQ:2 * NQ], axis=AX.X, op=ALU.add)

    # user-side tanh + norm + ehT: gated behind the es chain (gate1)
    btc_u = sb.tile([D, N_USER], F32, tag="btc_u")
    nc.scalar.activation(btc_u[:, 0:512], btb["u0"][:], AF.Copy,
                         scale=gate1[:, 0:1])
    nc.scalar.activation(btc_u[:, 512:N_USER], btb["u1"][:], AF.Copy,
                         scale=gate1[:, 0:1])
    if STN_FP8:
        btv_u = sb.tile([D, N_USER], F32, tag="btv_u")
        nc.gpsimd.tensor_mul(btv_u[:], btc_u[:], pk_sb[:, 0:N_USER])
        cut_in = btv_u[:]
    else:
        cut_in = btc_u[:]
    nc.scalar.activation(cut[:], cut_in, AF.Tanh, bias=b2)
    sqc = scr.tile([D, N_USER], F32, tag="sqc")
    nc.scalar.activation(sqc[:], cut[:], AF.Square, accum_out=rc[:, 1:2])
    nc.vector.reciprocal(rc[:, 1:2], rc[:, 1:2])
    rcu = rc[:, 3:4]
    nc.scalar.activation(rcu, rc[:, 1:2], AF.Sqrt)
    ecu = sb.tile([D, N_USER], F32, tag="ecu")
    nc.scalar.activation(ecu[:], cut[:], AF.Copy, scale=rcu)
    eht = sb.tile([D, N_USER], BF16, tag="eht")
    nc.gpsimd.tensor_add(eht[:], ecu[:], par_sb)

    # ---- all-reduce BN stats ([64,2]) ----
    st_in = dram.tile([D, 2], F32, tag="cc_in")
    st_out = dram.tile([D, 2], F32, tag="cc_out")
    nc.sync.dma_start(st_in[:], stats[:])

    if collective:
        nc.gpsimd.collective_compute(
            "AllReduce", mybir.AluOpType.add,
            replica_groups=[list(range(NCORES))],
            ins=[st_in.opt()], outs=[st_out.opt()])
    else:
        nc.sync.dma_start(st_out[:], st_in[:])
    ast = sb.tile([D, 2], F32, tag="ast")
    nc.sync.dma_start(ast[:], st_out[:])

    # ---- PE warm-filler bridges the collective window.  The moving
    # tensor is `cut` (ready only after the es chain) so fillers cannot
    # preempt the es2n/esy matmuls. ----
    for g in range(N_FILL_GROUPS):
        warm = psT.tile([D, QW], F32, tag="oL", name=f"warm{g}")
        for w in range(6):
            nc.tensor.matmul(warm[:], ones[:D, :], cut[:, 0:QW],
                             start=(w == 0), stop=(w == 5))

    # ---- BN coefficients (mlp_b cancels inside BN) ----
    bnt = sb.tile([D, 5], F32, tag="bnt")  # mu, var, mu2, s, t
    nc.vector.tensor_scalar_mul(bnt[:, 0:1], ast[:, 0:1], 1.0 / B)
    nc.vector.tensor_scalar_mul(bnt[:, 1:2], ast[:, 1:2], 1.0 / B)
    nc.vector.tensor_mul(bnt[:, 2:3], bnt[:, 0:1], bnt[:, 0:1])
    nc.vector.tensor_sub(bnt[:, 1:2], bnt[:, 1:2], bnt[:, 2:3])
    nc.scalar.activation(bnt[:, 2:3], bnt[:, 1:2], AF.Sqrt, bias=epst[:, 0:1])
    nc.vector.reciprocal(bnt[:, 2:3], bnt[:, 2:3])
    nc.vector.tensor_mul(bnt[:, 3:4], gam, bnt[:, 2:3])
    nc.vector.tensor_mul(bnt[:, 4:5], bnt[:, 0:1], bnt[:, 3:4])
    nc.vector.tensor_sub(bnt[:, 4:5], bet, bnt[:, 4:5])

    # ---- zbn (bf16, per out group) interleaved with out tiles;
    # PSUM->SBUF copies split 3 ways (DVE / Act / Pool) ----
    zbn = sb.tile([D, BS], BF16, tag="zbn")
    og = outp.tile([128, BS // 128, N_USER], BF16, tag="og", bufs=1)
    NT = BS // 128
    for bi in range(NT):
        if bi % 4 == 0:
            g = bi // 4
            nc.scalar.activation(zbn[:, g * QW:(g + 1) * QW],
                                 zt[:, g * QW:(g + 1) * QW], AF.Relu,
                                 bias=bnt[:, 4:5], scale=bnt[:, 3:4])
        OLW = 410
        oL = psT.tile([128, OLW], F32, tag="oL", name=f"oL{bi}")
        oR = psT.tile([128, N_USER - OLW], F32, tag="oR", name=f"oR{bi}")
        lhs = zbn[:, 128 * bi:128 * (bi + 1)]
        nc.tensor.matmul(oL[:], lhs, eht[:, 0:OLW], start=True, stop=True)
        nc.tensor.matmul(oR[:], lhs, eht[:, OLW:N_USER], start=True, stop=True)
        if bi % 2 == 0:
            nc.vector.tensor_copy(og[:, bi, 0:OLW], oL[:])
            nc.scalar.copy(og[:, bi, OLW:N_USER], oR[:])
        else:
            nc.scalar.copy(og[:, bi, 0:OLW], oL[:])
            nc.vector.tensor_copy(og[:, bi, OLW:N_USER], oR[:])
        if bi == 0 or bi == NT - 1:
            nc.sync.dma_start(out[:, bi:bi + 1, :], og[:, bi:bi + 1, :])
        elif bi % 2 == 0:
            nc.sync.dma_start(out[:, bi - 1:bi + 1, :], og[:, bi - 1:bi + 1, :])


_NC_CACHE = {}


def _get_nc():
    if "nc" not in _NC_CACHE:
        _NC_CACHE["nc"] = _build()
    return _NC_CACHE["nc"]


def _prep(inputs):
    import ml_dtypes
    bf16 = ml_dtypes.bfloat16
    f8 = ml_dtypes.float8_e4m3

    x_SH = np.asarray(inputs["x_SH"], dtype=np.int64)
    ei = np.asarray(inputs["edge_index_SH"])
    presc = np.asarray(inputs["prescription"], dtype=np.float32)
    SH_emb = np.asarray(inputs["SH_emb"], dtype=np.float32)
    W1 = np.asarray(inputs["W1"], dtype=np.float32)
    b1 = np.asarray(inputs["b1"], dtype=np.float32)
    W2 = np.asarray(inputs["W2"], dtype=np.float32)
    b2 = np.asarray(inputs["b2"], dtype=np.float32)
    mlp_W = np.asarray(inputs["mlp_W"], dtype=np.float32)
    gam = np.asarray(inputs["bn_gamma"], dtype=np.float32)
    bet = np.asarray(inputs["bn_beta"], dtype=np.float32)

    x1 = SH_emb[x_SH]                                       # (1195, 64)
    src = np.asarray(ei[0], dtype=np.int64)
    dst = np.asarray(ei[1], dtype=np.int64)
    stm = np.bincount(src * N_SH + dst, minlength=N_SH * N_SH).reshape(
        N_SH, N_SH).astype(np.float32)                      # S^T[s,d] counts
    cnt = stm.sum(axis=0)                                   # per-dst degree
    rdeg = 1.0 / np.maximum(cnt, 1.0)                       # (1195,)

    def chunked(a, width):
        # (1195, w) -> zero-pad rows to 1280 -> (128, 10, w)
        p = np.zeros((NPAD, width), dtype=a.dtype)
        p[:N_SH] = a
        return np.ascontiguousarray(
            p.reshape(NKC, 128, width).transpose(1, 0, 2))

    if STN_FP8:
        assert stm.max() <= 15, stm.max()     # fp8e4m3 integers exact to 16
        stn_p = chunked(stm.astype(f8), N_SH)
    else:
        stn_p = chunked((stm * rdeg[None, :]).astype(bf16), N_SH)
    x1w_full = x1 @ W1.T
    if DOUBLE_ROW:
        x1w_hi = x1w_full.astype(f8)
        x1w_lo = (x1w_full - x1w_hi.astype(np.float32)).astype(f8)
        hilo = np.stack([x1w_hi, x1w_lo], axis=1)        # (1195, 2, 64)
        x1w_p = chunked(hilo.reshape(N_SH, 2 * D), 2 * D).reshape(
            128, NKC, 2, D)
    else:
        x1w_p = chunked(x1w_full.astype(bf16), D)
    rdg_p = np.broadcast_to(rdeg[None, :], (D, N_SH)).astype(np.float32)

    nrm = np.sqrt((x1 * x1).sum(axis=1, keepdims=True))
    x1n = x1 / np.maximum(nrm, NORM_EPS)                    # (1195, 64)
    vec = np.stack([b1, b2, gam, bet], axis=1).astype(np.float32)
    par = x1n[:N_USER].T.astype(np.float32)                        # (64, 805)
    rdgb = np.ascontiguousarray(
        np.concatenate([rdg_p, vec], axis=1).astype(bf16))
    pv = np.ascontiguousarray(par.astype(bf16))
    xnm_full = x1n[N_USER:] @ mlp_W.T                       # (390, 64)
    xnm_pad = np.zeros((512, D), np.float32)
    xnm_pad[:N_ITEM] = xnm_full
    xnm_p = np.ascontiguousarray(
        xnm_pad.reshape(4, 128, D).transpose(1, 0, 2).astype(bf16))
    wts = np.ascontiguousarray(
        np.concatenate([W2.T, mlp_W.T], axis=1).astype(bf16))

    shared = {"stn": stn_p, "x1w": x1w_p, "rdgb": rdgb, "pv": pv,
              "xnm": xnm_p, "wts": wts}

    in_maps = []
    for c in range(NCORES):
        xt = presc[c * BS:(c + 1) * BS].T.astype(bf16)      # (390, 2048)
        x012 = np.ascontiguousarray(
            xt[:384].reshape(3, 128, BS).transpose(1, 0, 2))
        m = dict(shared)
        m["xp"] = x012
        m["xp3"] = np.ascontiguousarray(xt[384:390])
        in_maps.append(m)
    return in_maps


def _assemble(res):
    outs = []
    for c in range(NCORES):
        o = np.asarray(res.results[c]["out"])               # (128, 16, 805) bf16
        outs.append(o.transpose(1, 0, 2).reshape(BS, N_USER))
    return np.concatenate(outs, axis=0).astype(np.float32)


def kernel(**inputs):
    in_maps = _prep(inputs)
    nc = _get_nc()
    res = run_bass_kernel_spmd(nc, in_maps, list(range(NCORES)))
    return _assemble(res)


def run_traced(inputs, tmpdir=None):
    """Profiled run: returns (output, exec_time_ns, results_obj)."""
    in_maps = _prep(inputs)
    nc = _get_nc()
    res = run_bass_kernel_spmd(nc, in_maps, list(range(NCORES)),
                               trace=True, tmpdir=tmpdir)
    return _assemble(res), res.exec_time_ns, res


# revision 64
# speedup vs baseline: 1.1520x; 1.0035x over previous
"""Trainium2 Bass kernel for nn_KDHR (gnn_message_passing), v2.

Batch rows are sharded 2048/core across 8 cores; the tiny graph state is
replicated.  Per core:

  L1:  at = x1w^T @ StC.  StC is the edge-count matrix S^T shipped as
       EXACT small integers in fp8e4m3 (halves the dominant DMA); the
       1/deg column scale is applied on DVE afterwards, tanh(+b1) on Act.
       Each 805-aligned column block accumulates in its own PSUM tile so
       cross-engine readers never serialize at tile granularity.
  L2:  h1w pairs (two 128-chunks per PSUM bank, one copy per pair);
       bt = h1w^T @ StC with the ITEM block first - it gates the es chain.
  es:  es2 is never materialized: es2n_c = cit_c^T @ (sqrt(1/ssq)*mlpW^T)
       + x1n_item@mlpW^T (host-precomputed).  The single act-table switch
       (tanh -> sqrt set, 1283ns) fires right after the item tanh and
       overlaps the DVE column-norm reduce.  The user-side tanh/norm/ehT
       chain is readiness-gated (gate1) behind the es chain and fills
       Act/Pool slack during the esy era and the collective window.
  zt:  esy = es2n^T @ X per batch quarter; ONE fused DVE op forms
       zt = esy*(1/presum) AND the batch sum; sums of squares split
       Act (q0/q1) / DVE (q2/q3).  Presum quarters ride the oL PSUM
       rotation right after the L2 agg; recips chase on DVE.
  BN:  stats [64,2] all-reduced over 8 cores (DRAM staging hops + one
       AllReduce); PE warm-filler matmuls (moving tensor = cut, so they
       can never preempt real work) bridge the window and keep the
       tensor engine's p-state at full clock for the out stage.
  out: zbn = relu(zt*s+t) per quarter; per 128-row tile zbn^T @ ehT,
       PSUM->SBUF copies split DVE/Act, paired DMAs (single first/last)
       stream bf16 tiles out; the host concatenates and casts.

The one AllReduce is replaced by a local DRAM copy when collective=False
(the TimelineSim path test.py measures)."""

import os
import sys

for _p in ("/root/.axon_site", "/root/.axon_site/_ro/trn_rl_repo",
           "/root/.axon_site/_ro/pypackages", "/opt/trn_rl_repo", "/opt/pypackages"):
    if os.path.isdir(_p) and _p not in sys.path:
        sys.path.append(_p)

import numpy as np

import concourse.bass as bass
import concourse.mybir as mybir
import concourse.tile as tile
from concourse import bacc
from concourse.bass_utils import run_bass_kernel_spmd

N_USER, N_ITEM, N_SH, D = 805, 390, 1195, 64
B, NCORES = 16384, 8
BS = B // NCORES          # 2048 batch rows per core
NKC = 10                  # source-node chunks (1195 padded to 1280)
NPAD = NKC * 128
BN_EPS = 1e-5
NORM_EPS = 1e-12
F32 = mybir.dt.float32
BF16 = mybir.dt.bfloat16
F8 = mybir.dt.float8e4

STN_FP8 = True            # ship S^T as exact fp8 counts + device rdeg scale
DOUBLE_ROW = False         # fp8 DoubleRow aggs (hi/lo split keeps bf16 accuracy)
N_FILL_GROUPS = 6         # PE warm-filler groups of 6 matmuls in cc window

# agg col blocks, 805-aligned so the item block is a single PSUM tile
BLK_L1 = [("u0", 0, 512), ("u1", 512, 293), ("it", 805, 390)]
BLK_L2 = [("it", 805, 390), ("u0", 0, 512), ("u1", 512, 293)]
GR = ([(0, 2), (2, 2), (4, 2), (6, 2), (8, 2)] if DOUBLE_ROW else
      [(0, 1), (1, 2), (3, 2), (5, 2), (7, 2), (9, 1)])  # stn DMA groups
NQ = 4
QW = BS // NQ                                    # 512


def _build(collective=True):
    nc = bacc.Bacc("TRN2", target_bir_lowering=False, debug=False,
                   num_devices=NCORES)

    st_dt = F8 if STN_FP8 else BF16
    xp = nc.declare_dram_parameter("xp", [128, 3, BS], BF16, isOutput=False).ap()
    xp3 = nc.declare_dram_parameter("xp3", [6, BS], BF16, isOutput=False).ap()
    stn = nc.declare_dram_parameter("stn", [128, NKC, N_SH], st_dt, isOutput=False).ap()
    if DOUBLE_ROW:
        x1w = nc.declare_dram_parameter("x1w", [128, NKC, 2, D], F8,
                                        isOutput=False).ap()
    else:
        x1w = nc.declare_dram_parameter("x1w", [128, NKC, D], BF16,
                                        isOutput=False).ap()
    rdgb = nc.declare_dram_parameter("rdgb", [D, N_SH + 4], BF16, isOutput=False).ap()
    pv = nc.declare_dram_parameter("pv", [D, N_USER], BF16, isOutput=False).ap()
    xnm = nc.declare_dram_parameter("xnm", [128, 4, D], BF16, isOutput=False).ap()
    wts = nc.declare_dram_parameter("wts", [D, 2 * D], BF16, isOutput=False).ap()
    out = nc.declare_dram_parameter("out", [128, BS // 128, N_USER], BF16,
                                    isOutput=True).ap()

    from contextlib import ExitStack
    with tile.TileContext(nc) as tc, ExitStack() as ctx:
        pools = {
            "cst": ctx.enter_context(tc.tile_pool(name="cst", bufs=1)),
            "sb": ctx.enter_context(tc.tile_pool(name="sb", bufs=1)),
            "scr": ctx.enter_context(tc.tile_pool(name="scr", bufs=2)),
            "outp": ctx.enter_context(tc.tile_pool(name="outp", bufs=4)),
            "psA": ctx.enter_context(tc.tile_pool(name="psA", bufs=1, space="PSUM")),
            "psT": ctx.enter_context(tc.tile_pool(name="psT", bufs=2, space="PSUM")),
            "dram": ctx.enter_context(tc.tile_pool(name="dram", bufs=1, space="DRAM")),
        }
        _body(nc, tc, pools, xp, xp3, stn, x1w, rdgb, pv, xnm, wts,
              out, collective)

    nc.compile()
    return nc


def _body(nc, tc, P, xp, xp3, stn, x1w, rdgb, pv, xnm, wts, out,
          collective=True):
    AF = mybir.ActivationFunctionType
    ALU = mybir.AluOpType
    AX = mybir.AxisListType
    cst, sb, scr, outp = P["cst"], P["sb"], P["scr"], P["outp"]
    psA, psT, dram = P["psA"], P["psT"], P["dram"]
    st_dt = F8 if STN_FP8 else BF16

    # ---- constants + engine warm-up (no DMA deps) ----
    ones = cst.tile([128, D], BF16, tag="ones")
    nc.gpsimd.memset(ones[:], 1.0)
    epst = cst.tile([D, 1], F32, tag="epst")
    nc.vector.memset(epst[:], BN_EPS)
    warmact = cst.tile([D, 1], F32, tag="warmact")
    with tc.high_priority():
        nc.scalar.activation(warmact[:], epst[:], AF.Tanh)
    # PE warm touch: starts the p-state epoch early (borrows the oL slot)
    warmp = psT.tile([D, QW], F32, tag="oL", name="wp")
    nc.tensor.matmul(warmp[:, 0:D], ones[:], ones[:, :D], start=True, stop=True)

    # ---- DMAs, all via SP.  HWDGE descriptor-gen is a serialized shared
    # device and DMA transfers are serialized too, so issue order == need
    # order: stn chunk0 + x1w chunk0 gate L1's start. ----
    stg = []
    for gi, (g0, gn) in enumerate(GR):
        t = sb.tile([128, gn, N_SH], st_dt, tag=f"stn{gi}", name=f"stn{gi}")
        stg.append(t)
    if DOUBLE_ROW:
        x1w_sb = cst.tile([128, NKC, 2, D], F8, tag="x1w")
    else:
        x1w_sb = cst.tile([128, NKC, D], BF16, tag="x1w")
    k0n = 2 if DOUBLE_ROW else 1
    nc.sync.dma_start(stg[0][:], stn[:, 0:k0n, :])
    nc.sync.dma_start(x1w_sb[:, 0:k0n], x1w[:, 0:k0n])
    nc.sync.dma_start(x1w_sb[:, k0n:NKC], x1w[:, k0n:NKC])
    for gi, (g0, gn) in enumerate(GR[1:], start=1):
        nc.sync.dma_start(stg[gi][:], stn[:, g0:g0 + gn, :])

    wts_sb = cst.tile([D, 2 * D], BF16, tag="wts")
    nc.sync.dma_start(wts_sb[:], wts[:, :])
    w2b = wts_sb[:, 0:D]
    mwb = wts_sb[:, D:2 * D]
    pk_sb = cst.tile([D, N_SH + 4], BF16, tag="pk")  # rdeg rows | b1 b2 g b
    nc.sync.dma_start(pk_sb[:], rdgb[:, :])
    b1 = pk_sb[:, N_SH + 0:N_SH + 1]
    b2 = pk_sb[:, N_SH + 1:N_SH + 2]
    gam = pk_sb[:, N_SH + 2:N_SH + 3]
    bet = pk_sb[:, N_SH + 3:N_SH + 4]
    # X in 3 chunk-slices so the presum matmuls can start as each lands
    # (fills the PE gap between L1's end and the L2 agg)
    X = sb.tile([128, 3, BS], BF16, tag="X")
    nc.sync.dma_start(X[:, 0:1, :], xp[:, 0:1, :])
    nc.sync.dma_start(X[:, 1:2, :], xp[:, 1:2, :])
    X3 = sb.tile([6, BS], BF16, tag="X3")
    nc.sync.dma_start(X3[:], xp3[:, :])
    nc.sync.dma_start(X[:, 2:3, :], xp[:, 2:3, :])
    pv_sb = cst.tile([D, N_USER], BF16, tag="pv")      # x1nuT
    nc.sync.dma_start(pv_sb[:], pv[:, :])
    par_sb = pv_sb[:, 0:N_USER]
    xnm_sb = cst.tile([128, 4, D], BF16, tag="xnm")
    nc.sync.dma_start(xnm_sb[:], xnm[:, :, :])

    def st_chunk(k, c0, cn):
        for gi, (g0, gn) in enumerate(GR):
            if g0 <= k < g0 + gn:
                return stg[gi][:, k - g0, c0:c0 + cn]
        raise AssertionError(k)

    def st_pair(c, c0, cn):
        # [128, 2, cn] moving AP for DoubleRow chunk-pair c
        gi = c  # GR is [(0,2),(2,2),...] in DR mode
        return stg[gi][:, 0:2, c0:c0 + cn]

    # ---- L1: at = x1w^T @ StC, chunk-pipelined behind the stn DMAs.
    # Each col block accumulates in its OWN PSUM tile: readers on
    # different engines then never serialize at tile granularity. ----
    atb = {}
    for bn, c0, cn in BLK_L1:
        atb[bn] = psA.tile([D, cn], F32, tag=f"ag_{bn}", name=f"at_{bn}")
    if DOUBLE_ROW:
        DR = mybir.MatmulPerfMode.DoubleRow
        for c in range(NKC // 2):
            for hl in (0, 1):
                for bn, c0, cn in BLK_L1:
                    nc.tensor.matmul(atb[bn][:],
                                     x1w_sb[:, 2 * c:2 * c + 2, hl, :],
                                     st_pair(c, c0, cn), perf_mode=DR,
                                     start=(c == 0 and hl == 0),
                                     stop=(c == NKC // 2 - 1 and hl == 1))
    else:
        for k in range(NKC):
            for bn, c0, cn in BLK_L1:
                nc.tensor.matmul(atb[bn][:], x1w_sb[:, k, :], st_chunk(k, c0, cn),
                                 start=(k == 0), stop=(k == NKC - 1))
    h1t = sb.tile([D, NPAD], BF16, tag="h1t")
    nc.vector.memset(h1t[:, N_SH:NPAD], 0.0)
    if STN_FP8:
        atv = {bn: sb.tile([D, cn], F32, tag=f"atv_{bn}", name=f"atv_{bn}")
               for bn, c0, cn in BLK_L1}
        # first half of u0 split out: h1w pair0 (cols 0:256) unblocks the
        # whole L2 chain ~0.8us earlier
        nc.vector.tensor_mul(atv["u0"][:, 0:256], atb["u0"][:, 0:256],
                             pk_sb[:, 0:256])
        nc.scalar.activation(h1t[:, 0:256], atv["u0"][:, 0:256], AF.Tanh,
                             bias=b1)
        nc.vector.tensor_mul(atv["u0"][:, 256:512], atb["u0"][:, 256:512],
                             pk_sb[:, 256:512])
        nc.scalar.activation(h1t[:, 256:512], atv["u0"][:, 256:512], AF.Tanh,
                             bias=b1)
        nc.vector.tensor_mul(atv["u1"][:], atb["u1"][:], pk_sb[:, 512:805])
        nc.scalar.activation(h1t[:, 512:805], atv["u1"][:], AF.Tanh, bias=b1)
        nc.vector.tensor_mul(atv["it"][:], atb["it"][:], pk_sb[:, 805:N_SH])
        nc.scalar.activation(h1t[:, 805:N_SH], atv["it"][:], AF.Tanh, bias=b1)
    else:
        for bn, c0, cn in BLK_L1:
            nc.scalar.activation(h1t[:, c0:c0 + cn], atb[bn][:], AF.Tanh,
                                 bias=b1)

    # ---- L2 prep: h1w pairs (two matmuls into one PSUM bank, ONE copy op
    # per pair -> no tile-granular write/read interleaving stalls) ----
    h1w = []
    h1wl = []
    for p in range(NKC // 2):
        tp = psT.tile([128, 2, D], F32, tag="oL", name=f"h1wp{p}")
        for j in range(2):
            k = 2 * p + j
            nc.tensor.matmul(tp[:, j, :], h1t[:, 128 * k:128 * (k + 1)], w2b,
                             start=True, stop=True)
        if DOUBLE_ROW:
            hb = sb.tile([128, 2, D], F8, tag=f"h1w{p}", name=f"h1w{p}")
            nc.scalar.copy(hb[:], tp[:])
            lb = sb.tile([128, 2, D], F8, tag=f"h1wl{p}", name=f"h1wl{p}")
            nc.vector.scalar_tensor_tensor(lb[:], tp[:], 1.0, hb[:],
                                           ALU.bypass, ALU.subtract)
            h1wl.append(lb)
        else:
            hb = sb.tile([128, 2, D], BF16, tag=f"h1w{p}", name=f"h1w{p}")
            if p % 2 == 0:
                nc.vector.tensor_copy(hb[:], tp[:])
            else:
                nc.scalar.copy(hb[:], tp[:])
        h1w.append(hb)

    # ---- L2: bt = h1w^T @ StC (item block first: it gates the es chain) ----
    btb = {}
    for bn, c0, cn in BLK_L2:
        btb[bn] = psA.tile([D, cn], F32, tag=f"ag_{bn}", name=f"bt_{bn}")
    if DOUBLE_ROW:
        DR = mybir.MatmulPerfMode.DoubleRow
        for c in range(NKC // 2):
            for hl, hsrc in ((0, h1w), (1, h1wl)):
                for bn, c0, cn in BLK_L2:
                    nc.tensor.matmul(btb[bn][:], hsrc[c][:],
                                     st_pair(c, c0, cn), perf_mode=DR,
                                     start=(c == 0 and hl == 0),
                                     stop=(c == NKC // 2 - 1 and hl == 1))
    else:
        for k in range(NKC):
            for bn, c0, cn in BLK_L2:
                nc.tensor.matmul(btb[bn][:], h1w[k // 2][:, k % 2, :],
                                 st_chunk(k, c0, cn),
                                 start=(k == 0), stop=(k == NKC - 1))


    # ---- presum quarters (post-agg PE slot); recips split in halves so
    # the es-chain's DVE ops can slot between them ----
    rp_sb = sb.tile([D, BS], F32, tag="rp_sb")
    for q in range(NQ):
        t = psT.tile([D, QW], F32, tag="oR", name=f"ps{q}")
        nc.tensor.matmul(t[:], ones[:], X[:, 0, q * QW:(q + 1) * QW],
                         start=True, stop=False)
        nc.tensor.matmul(t[:], ones[:], X[:, 1, q * QW:(q + 1) * QW],
                         start=False, stop=False)
        nc.tensor.matmul(t[:], ones[:6, :], X3[:, q * QW:(q + 1) * QW],
                         start=False, stop=False)
        nc.tensor.matmul(t[:], ones[:], X[:, 2, q * QW:(q + 1) * QW],
                         start=False, stop=True)
        nc.vector.reciprocal(rp_sb[:, q * QW:(q + 1) * QW], t[:])

    # ---- item-side scale + tanh (gates the es chain) ----
    cit = sb.tile([D, 512], BF16, tag="cit")   # padded to 512 items
    nc.vector.memset(cit[:, N_ITEM:512], 0.0)
    cut = sb.tile([D, N_USER], BF16, tag="cut")
    rc = sb.tile([D, 4], F32, tag="rc")  # ssq_i, ssq_u, scale_i, scale_u
    sqj = scr.tile([D, 512], F32, tag="sqj")  # reduce junk
    if STN_FP8:
        btv_i = sb.tile([D, N_ITEM], F32, tag="btv_i")
        nc.vector.tensor_mul(btv_i[:], btb["it"][:], pk_sb[:, N_USER:N_SH])
        nc.scalar.activation(cit[:, 0:N_ITEM], btv_i[:], AF.Tanh, bias=b2)
    else:
        nc.scalar.activation(cit[:, 0:N_ITEM], btb["it"][:], AF.Tanh, bias=b2)

    # item col-norm on DVE: scale_i = sqrt(1/sumsq).  The Act table switch
    # (tanh -> sqrt set, 1283ns) rides on the rci Sqrt; the user tanh is
    # issued much later so it doesn't squeeze in front of the switch.
    nc.vector.scalar_tensor_tensor(
        sqj[:, 0:N_ITEM], cit[:, 0:N_ITEM], 1.0, cit[:, 0:N_ITEM],
        ALU.bypass, ALU.mult, accum_out=rc[:, 0:1])
    nc.vector.reciprocal(rc[:, 0:1], rc[:, 0:1])
    nc.scalar.activation(rc[:, 2:3], rc[:, 0:1], AF.Sqrt)
    mwbs = sb.tile([D, D], BF16, tag="mwbs")
    nc.scalar.activation(mwbs[:], mwb, AF.Copy, scale=rc[:, 2:3])
    # readiness gate: the user-side scale (and thus cut's tanh) must not
    # become schedulable before mwbs, or the Act queue runs it ahead of the
    # act-table switch and delays the es chain by ~2us
    gate1 = sb.tile([D, 1], F32, tag="gate1")
    nc.vector.tensor_scalar_mul(gate1[:], rc[:, 2:3], 0.0)
    nc.vector.tensor_scalar_add(gate1[:], gate1[:], 1.0)

    # ---- es2n pairs: es2n_c = cit_c^T @ mwbs + x1n_item@mlpW.T
    # (pair1 borrows the oR slot so the two pairs don't serialize on the
    # single tr bank; adds split DVE/Pool) ----
    es2n = []
    for p in range(2):
        tag = "tr" if p == 0 else "oR"
        bufs = {"bufs": 1} if p == 0 else {}
        tp = psT.tile([128, 2, D], F32, tag=tag, name=f"es2p{p}", **bufs)
        for j in range(2):
            c = 2 * p + j
            nc.tensor.matmul(tp[:, j, :], cit[:, 128 * c:128 * (c + 1)], mwbs[:],
                             start=True, stop=True)
        eb = sb.tile([128, 2, D], BF16, tag=f"es2n{p}", name=f"es2n{p}")
        nc.vector.tensor_add(eb[:], tp[:], xnm_sb[:, 2 * p:2 * p + 2, :])
        es2n.append(eb)


    # ---- esy quarters -> fused zt = esy*rp with batch-sum accum ----
    zt = sb.tile([D, BS], F32, tag="zt")
    s12 = sb.tile([D, 2 * NQ], F32, tag="s12")
    XC = [X[:, 0, :], X[:, 1, :], X[:, 2, :], X3]
    KN = [128, 128, 128, 6]
    for q in range(NQ):
        t = psT.tile([D, QW], F32, tag="oR", name=f"esy{q}")
        for c in range(4):
            eb = es2n[c // 2][:KN[c], c % 2, :]
            nc.tensor.matmul(t[:], eb, XC[c][:KN[c], q * QW:(q + 1) * QW],
                             start=(c == 0), stop=(c == 3))
        ztq = zt[:, q * QW:(q + 1) * QW]
        nc.vector.scalar_tensor_tensor(
            ztq, t[:], 1.0, rp_sb[:, q * QW:(q + 1) * QW],
            ALU.bypass, ALU.mult, accum_out=s12[:, q:q + 1])
        if q < 2:
            sq = scr.tile([D, QW], F32, tag="sq")
            nc.scalar.activation(sq[:], ztq, AF.Square,
                                 accum_out=s12[:, NQ + q:NQ + q + 1])
        else:
            sq3 = scr.tile([D, QW], F32, tag="sq")
            nc.vector.scalar_tensor_tensor(
                sq3[:], ztq, 1.0, ztq, ALU.bypass, ALU.mult,
                accum_out=s12[:, NQ + q:NQ + q + 1])

    stats = sb.tile([D, 2], F32, tag="stats")
    nc.vector.tensor_reduce(stats[:, 0:1], s12[:, 0:NQ], axis=AX.X, op=ALU.add)
    nc.vector.tensor_reduce(stats[:, 1:2], s12[:, NQ:2 * NQ], axis=AX.X, op=ALU.add)

    # user-side tanh + norm + ehT: gated behind the es chain (gate1)
    btc_u = sb.tile([D, N_USER], F32, tag="btc_u")
    nc.scalar.activation(btc_u[:, 0:512], btb["u0"][:], AF.Copy,
                         scale=gate1[:, 0:1])
    nc.scalar.activation(btc_u[:, 512:N_USER], btb["u1"][:], AF.Copy,
                         scale=gate1[:, 0:1])
    if STN_FP8:
        btv_u = sb.tile([D, N_USER], F32, tag="btv_u")
        nc.gpsimd.tensor_mul(btv_u[:], btc_u[:], pk_sb[:, 0:N_USER])
        cut_in = btv_u[:]
    else:
        cut_in = btc_u[:]
    nc.scalar.activation(cut[:], cut_in, AF.Tanh, bias=b2)
    sqc = scr.tile([D, N_USER], F32, tag="sqc")
    nc.scalar.activation(sqc[:], cut[:], AF.Square, accum_out=rc[:, 1:2])
    nc.vector.reciprocal(rc[:, 1:2], rc[:, 1:2])
    rcu = rc[:, 3:4]
    nc.scalar.activation(rcu, rc[:, 1:2], AF.Sqrt)
    ecu = sb.tile([D, N_USER], F32, tag="ecu")
    nc.scalar.activation(ecu[:], cut[:], AF.Copy, scale=rcu)
    eht = sb.tile([D, N_USER], BF16, tag="eht")
    nc.gpsimd.tensor_add(eht[:], ecu[:], par_sb)

    # ---- all-reduce BN stats ([64,2]) ----
    st_in = dram.tile([D, 2], F32, tag="cc_in")
    st_out = dram.tile([D, 2], F32, tag="cc_out")
    nc.sync.dma_start(st_in[:], stats[:])

    if collective:
        nc.gpsimd.collective_compute(
            "AllReduce", mybir.AluOpType.add,
            replica_groups=[list(range(NCORES))],
            ins=[st_in.opt()], outs=[st_out.opt()])
    else:
        nc.sync.dma_start(st_out[:], st_in[:])
    ast = sb.tile([D, 2], F32, tag="ast")
    nc.sync.dma_start(ast[:], st_out[:])

    # ---- PE warm-filler bridges the collective window.  The moving
    # tensor is `cut` (ready only after the es chain) so fillers cannot
    # preempt the es2n/esy matmuls. ----
    for g in range(N_FILL_GROUPS):
        warm = psT.tile([D, QW], F32, tag="oL", name=f"warm{g}")
        for w in range(6):
            nc.tensor.matmul(warm[:], ones[:D, :], cut[:, 0:QW],
                             start=(w == 0), stop=(w == 5))

    # ---- BN coefficients (mlp_b cancels inside BN) ----
    bnt = sb.tile([D, 5], F32, tag="bnt")  # mu, var, mu2, s, t
    nc.vector.tensor_scalar_mul(bnt[:, 0:1], ast[:, 0:1], 1.0 / B)
    nc.vector.tensor_scalar_mul(bnt[:, 1:2], ast[:, 1:2], 1.0 / B)
    nc.vector.tensor_mul(bnt[:, 2:3], bnt[:, 0:1], bnt[:, 0:1])
    nc.vector.tensor_sub(bnt[:, 1:2], bnt[:, 1:2], bnt[:, 2:3])
    nc.scalar.activation(bnt[:, 2:3], bnt[:, 1:2], AF.Sqrt, bias=epst[:, 0:1])
    nc.vector.reciprocal(bnt[:, 2:3], bnt[:, 2:3])
    nc.vector.tensor_mul(bnt[:, 3:4], gam, bnt[:, 2:3])
    nc.vector.tensor_mul(bnt[:, 4:5], bnt[:, 0:1], bnt[:, 3:4])
    nc.vector.tensor_sub(bnt[:, 4:5], bet, bnt[:, 4:5])

    # ---- zbn (bf16, per out group) interleaved with out tiles;
    # PSUM->SBUF copies split 3 ways (DVE / Act / Pool) ----
    zbn = sb.tile([D, BS], BF16, tag="zbn")
    og = outp.tile([128, BS // 128, N_USER], BF16, tag="og", bufs=1)
    NT = BS // 128
    for bi in range(NT):
        if bi % 4 == 0:
            g = bi // 4
            nc.scalar.activation(zbn[:, g * QW:(g + 1) * QW],
                                 zt[:, g * QW:(g + 1) * QW], AF.Relu,
                                 bias=bnt[:, 4:5], scale=bnt[:, 3:4])
        OLW = 410
        oL = psT.tile([128, OLW], F32, tag="oL", name=f"oL{bi}")
        oR = psT.tile([128, N_USER - OLW], F32, tag="oR", name=f"oR{bi}")
        lhs = zbn[:, 128 * bi:128 * (bi + 1)]
        nc.tensor.matmul(oL[:], lhs, eht[:, 0:OLW], start=True, stop=True)
        nc.tensor.matmul(oR[:], lhs, eht[:, OLW:N_USER], start=True, stop=True)
        if bi % 2 == 0:
            nc.vector.tensor_copy(og[:, bi, 0:OLW], oL[:])
            nc.scalar.copy(og[:, bi, OLW:N_USER], oR[:])
        else:
            nc.scalar.copy(og[:, bi, 0:OLW], oL[:])
            nc.vector.tensor_copy(og[:, bi, OLW:N_USER], oR[:])
        if bi == 0 or bi == NT - 1:
            nc.sync.dma_start(out[:, bi:bi + 1, :], og[:, bi:bi + 1, :])
        elif bi % 2 == 0:
            nc.sync.dma_start(out[:, bi - 1:bi + 1, :], og[:, bi - 1:bi + 1, :])


_NC_CACHE = {}


def _get_nc():
    if "nc" not in _NC_CACHE:
        _NC_CACHE["nc"] = _build()
    return _NC_CACHE["nc"]


def _prep(inputs):
    import ml_dtypes
    bf16 = ml_dtypes.bfloat16
    f8 = ml_dtypes.float8_e4m3

    x_SH = np.asarray(inputs["x_SH"], dtype=np.int64)
    ei = np.asarray(inputs["edge_index_SH"])
    presc = np.asarray(inputs["prescription"], dtype=np.float32)
    SH_emb = np.asarray(inputs["SH_emb"], dtype=np.float32)
    W1 = np.asarray(inputs["W1"], dtype=np.float32)
    b1 = np.asarray(inputs["b1"], dtype=np.float32)
    W2 = np.asarray(inputs["W2"], dtype=np.float32)
    b2 = np.asarray(inputs["b2"], dtype=np.float32)
    mlp_W = np.asarray(inputs["mlp_W"], dtype=np.float32)
    gam = np.asarray(inputs["bn_gamma"], dtype=np.float32)
    bet = np.asarray(inputs["bn_beta"], dtype=np.float32)

    x1 = SH_emb[x_SH]                                       # (1195, 64)
    src = np.asarray(ei[0], dtype=np.int64)
    dst = np.asarray(ei[1], dtype=np.int64)
    stm = np.bincount(src * N_SH + dst, minlength=N_SH * N_SH).reshape(
        N_SH, N_SH).astype(np.float32)                      # S^T[s,d] counts
    cnt = stm.sum(axis=0)                                   # per-dst degree
    rdeg = 1.0 / np.maximum(cnt, 1.0)                       # (1195,)

    def chunked(a, width):
        # (1195, w) -> zero-pad rows to 1280 -> (128, 10, w)
        p = np.zeros((NPAD, width), dtype=a.dtype)
        p[:N_SH] = a
        return np.ascontiguousarray(
            p.reshape(NKC, 128, width).transpose(1, 0, 2))

    if STN_FP8:
        assert stm.max() <= 15, stm.max()     # fp8e4m3 integers exact to 16
        stn_p = chunked(stm.astype(f8), N_SH)
    else:
        stn_p = chunked((stm * rdeg[None, :]).astype(bf16), N_SH)
    x1w_full = x1 @ W1.T
    if DOUBLE_ROW:
        x1w_hi = x1w_full.astype(f8)
        x1w_lo = (x1w_full - x1w_hi.astype(np.float32)).astype(f8)
        hilo = np.stack([x1w_hi, x1w_lo], axis=1)        # (1195, 2, 64)
        x1w_p = chunked(hilo.reshape(N_SH, 2 * D), 2 * D).reshape(
            128, NKC, 2, D)
    else:
        x1w_p = chunked(x1w_full.astype(bf16), D)
    rdg_p = np.broadcast_to(rdeg[None, :], (D, N_SH)).astype(np.float32)

    nrm = np.sqrt((x1 * x1).sum(axis=1, keepdims=True))
    x1n = x1 / np.maximum(nrm, NORM_EPS)                    # (1195, 64)
    vec = np.stack([b1, b2, gam, bet], axis=1).astype(np.float32)
    par = x1n[:N_USER].T.astype(np.float32)                        # (64, 805)
    rdgb = np.ascontiguousarray(
        np.concatenate([rdg_p, vec], axis=1).astype(bf16))
    pv = np.ascontiguousarray(par.astype(bf16))
    xnm_full = x1n[N_USER:] @ mlp_W.T                       # (390, 64)
    xnm_pad = np.zeros((512, D), np.float32)
    xnm_pad[:N_ITEM] = xnm_full
    xnm_p = np.ascontiguousarray(
        xnm_pad.reshape(4, 128, D).transpose(1, 0, 2).astype(bf16))
    wts = np.ascontiguousarray(
        np.concatenate([W2.T, mlp_W.T], axis=1).astype(bf16))

    shared = {"stn": stn_p, "x1w": x1w_p, "rdgb": rdgb, "pv": pv,
              "xnm": xnm_p, "wts": wts}

    in_maps = []
    for c in range(NCORES):
        xt = presc[c * BS:(c + 1) * BS].T.astype(bf16)      # (390, 2048)
        x012 = np.ascontiguousarray(
            xt[:384].reshape(3, 128, BS).transpose(1, 0, 2))
        m = dict(shared)
        m["xp"] = x012
        m["xp3"] = np.ascontiguousarray(xt[384:390])
        in_maps.append(m)
    return in_maps


def _assemble(res):
    outs = []
    for c in range(NCORES):
        o = np.asarray(res.results[c]["out"])               # (128, 16, 805) bf16
        outs.append(o.transpose(1, 0, 2).reshape(BS, N_USER))
    return np.concatenate(outs, axis=0).astype(np.float32)


def kernel(**inputs):
    in_maps = _prep(inputs)
    nc = _get_nc()
    res = run_bass_kernel_spmd(nc, in_maps, list(range(NCORES)))
    return _assemble(res)


def run_traced(inputs, tmpdir=None):
    """Profiled run: returns (output, exec_time_ns, results_obj)."""
    in_maps = _prep(inputs)
    nc = _get_nc()
    res = run_bass_kernel_spmd(nc, in_maps, list(range(NCORES)),
                               trace=True, tmpdir=tmpdir)
    return _assemble(res), res.exec_time_ns, res
